# revision 23
# baseline (speedup 1.0000x reference)
"""Trainium2 Bass kernel for AttentionConditionGenerator.

Reference computation (per row b of B=16384):
    kv = [h_u_cross[b], h_u_target[b]]            # (2, 1024)
    q  = dom_movie @ w_q.T + b_q                  # fixed across rows
    scores = (q/8) . k[s],  attn = softmax_2(scores)
    ctx = attn0*v0 + attn1*v1 ; y = ctx @ w_o.T + b_o
    x = LN1(dom_movie + y); h = gelu(x @ w1.T + b1)
    out = LN2(x + h @ w2.T + b2)

Algebraic folding (host, fp64, exact):
  - q row-independent -> scores fold to A @ d with d = xc - xt;
    attn0 = sigmoid(A @ d) (b_k cancels in the 2-way softmax).
  - ctx = v_t + attn0 * v_d, v_t = w_v@xt + b_v, v_d = w_v@d.
  - LN1 centering folded into the weights: with C = I - 11^T/D,
        y' = C@y = (C@w_o@w_v)@xt + (C@w_o)@(attn0*(w_v@d)) + C@bod2
    so y' arrives centered and LN1 reduces to a per-row scale:
        x = y' * rsqrt(mean(y'^2) + eps)
    x is exactly column-centered too, so LN2 sees a centered residual.

Device mapping: batch split over 8 cores (2048 rows each). Activations stay
feature-major (features on partitions) end-to-end; LN1 runs feature-major
(variance via ones-vector matmuls + a 1-row broadcast matmul), LN2 runs
row-major after cheap bf16 transposes of the final residual. All matmuls are
bf16 with fp32 PSUM accumulation.
"""

import numpy as np
import ml_dtypes

try:
    import concourse.bass as bass
except ImportError:  # pragma: no cover - path setup for fresh environments
    import sys

    for _p in ("/opt/trn_rl_repo", "/root/.axon_site/_ro/trn_rl_repo"):
        if _p not in sys.path:
            sys.path.insert(0, _p)
    import concourse.bass as bass

import concourse.mybir as mybir
import concourse.tile as tile
from concourse import bacc
from concourse.bass_utils import run_bass_kernel_spmd
from concourse.masks import make_identity

F32 = mybir.dt.float32
BF16 = mybir.dt.bfloat16
NPBF16 = ml_dtypes.bfloat16

D = 1024
H = 16
HD = 64
FFN = 4096
EPS = 1e-5
N_CORES = 8
B_TOTAL = 16384
B_CORE = B_TOTAL // N_CORES  # 2048

KT = D // 128  # 8 feature k-tiles
MT = D // 128  # 8 output m-tiles
FMT = FFN // 128  # 32 FFN m-tiles
NMG = 4  # host-side FFN1 m-group axis (1024 cols each)

AF = mybir.ActivationFunctionType


def build_program(b_core, r_blk, trivial_ln1, trivial_ln2):
    """Build and compile the per-core Bass program."""
    nb = b_core // r_blk  # row blocks
    ni = r_blk // 128  # 128-row subtiles per block
    N = r_blk  # matmul moving (free) dim

    nc = bacc.Bacc("TRN2", target_bir_lowering=False)

    # ---- DRAM I/O ------------------------------------------------------
    xtb_d = nc.dram_tensor("xtb", [b_core, D], BF16, kind="ExternalInput")
    db_d = nc.dram_tensor("db", [b_core, D], BF16, kind="ExternalInput")
    wv_d = nc.dram_tensor("wvT", [128, KT, D], BF16, kind="ExternalInput")
    wm_d = nc.dram_tensor("wmT", [128, KT, D], BF16, kind="ExternalInput")
    wo_d = nc.dram_tensor("woT", [128, KT, D], BF16, kind="ExternalInput")
    w1_d = nc.dram_tensor("w1P", [128, NMG, KT, D], BF16, kind="ExternalInput")
    w2_d = nc.dram_tensor("w2P", [128, MT, FMT, 128], BF16, kind="ExternalInput")
    at_d = nc.dram_tensor("AT", [128, KT, H], BF16, kind="ExternalInput")
    e_d = nc.dram_tensor("E", [H, MT, 128], BF16, kind="ExternalInput")
    bod_d = nc.dram_tensor("bodC", [128, MT], F32, kind="ExternalInput")
    b1_d = nc.dram_tensor("b1p", [128, FMT], F32, kind="ExternalInput")
    b2_d = nc.dram_tensor("b2p", [128, MT], F32, kind="ExternalInput")
    if not trivial_ln1:
        g1_d = nc.dram_tensor("g1p", [128, MT], F32, kind="ExternalInput")
        c1_d = nc.dram_tensor("c1p", [128, MT], F32, kind="ExternalInput")
    if not trivial_ln2:
        g2_d = nc.dram_tensor("g2", [D], F32, kind="ExternalInput")
        c2_d = nc.dram_tensor("c2", [D], F32, kind="ExternalInput")
    out_d = nc.dram_tensor("out", [b_core, D], F32, kind="ExternalOutput")

    def bcast_ap(dram):
        # [D] dram vector -> [128, D] partition-broadcast access pattern
        return bass.AP(tensor=dram, offset=0, ap=[[0, 128], [1, D]])

    with tile.TileContext(nc) as tc:
        with (
            tc.tile_pool(name="consts", bufs=1) as consts,
            tc.tile_pool(name="wres", bufs=1) as wres,
            tc.tile_pool(name="w1s", bufs=2) as w1pool,
            tc.tile_pool(name="w2s", bufs=2) as w2pool,
            tc.tile_pool(name="fm", bufs=1) as fm,
            tc.tile_pool(name="act", bufs=1) as am,
            tc.tile_pool(name="work", bufs=3) as work,
            tc.tile_pool(name="stats", bufs=2) as st,
            tc.tile_pool(name="rm", bufs=2) as rm,
            tc.tile_pool(name="ps_mm", bufs=3, space="PSUM") as ps_mm,
            tc.tile_pool(name="ps_ab", bufs=2, space="PSUM") as ps_ab,
            tc.tile_pool(name="ps_misc", bufs=2, space="PSUM") as ps_misc,
            tc.tile_pool(name="ps_tr", bufs=1, space="PSUM") as ps_tr,
        ):
            # ---- constants ------------------------------------------
            at_s = consts.tile([128, KT, H], BF16)
            nc.sync.dma_start(at_s, at_d[:, :, :])
            ident = consts.tile([128, 128], BF16)
            make_identity(nc, ident)
            e_s = consts.tile([H, MT, 128], BF16)
            nc.sync.dma_start(e_s, e_d[:, :, :])
            bod_s = consts.tile([128, MT], F32)
            nc.sync.dma_start(bod_s, bod_d[:, :])
            b1_s = consts.tile([128, FMT], F32)
            nc.sync.dma_start(b1_s, b1_d[:, :])
            b2_s = consts.tile([128, MT], F32)
            nc.sync.dma_start(b2_s, b2_d[:, :])
            ones_col = consts.tile([128, 1], BF16)
            nc.vector.memset(ones_col, 1.0)
            ones_row = consts.tile([1, 128], BF16)
            nc.vector.memset(ones_row, 1.0)
            eps_s = consts.tile([128, 1], F32)
            nc.vector.memset(eps_s, EPS)
            eps1 = consts.tile([1, 1], F32)
            nc.vector.memset(eps1, EPS)
            g1_s = c1_s = g2_s = c2_s = None
            if not trivial_ln1:
                g1_s = consts.tile([128, MT], F32)
                nc.sync.dma_start(g1_s, g1_d[:, :])
                c1_s = consts.tile([128, MT], F32)
                nc.sync.dma_start(c1_s, c1_d[:, :])
            if not trivial_ln2:
                g2_s = consts.tile([128, D], F32)
                nc.gpsimd.dma_start(g2_s, bcast_ap(g2_d))
                c2_s = consts.tile([128, D], F32)
                nc.gpsimd.dma_start(c2_s, bcast_ap(c2_d))

            def layernorm_rm(y_i, out_tile, g_s, c_s, last=False):
                """Row-major LayerNorm of y_i [128, D] f32 -> out_tile.
                last=True puts the normalize on DVE (the Pool impl is
                ~2x slower, which only matters on the kernel tail)."""
                stt = st.tile([128, 2, 6], F32, tag="bnst")
                nc.vector.bn_stats(stt[:, 0, :], y_i[:, 0:512])
                nc.vector.bn_stats(stt[:, 1, :], y_i[:, 512:1024])
                mv = st.tile([128, 2], F32, tag="bnmv")
                nc.vector.bn_aggr(mv, stt)
                std = st.tile([128, 1], F32, tag="bnstd")
                nc.scalar.activation(std, mv[:, 1:2], AF.Sqrt, bias=eps_s)
                rstd = st.tile([128, 1], F32, tag="bnrstd")
                nc.vector.reciprocal(rstd, std)
                eng = nc.vector if last else nc.gpsimd
                if g_s is None:
                    eng.tensor_scalar(
                        out_tile, y_i, mv[:, 0:1], rstd,
                        op0=mybir.AluOpType.subtract, op1=mybir.AluOpType.mult)
                else:
                    xn = st.tile([128, D], F32, tag="bnxn")
                    nc.gpsimd.tensor_scalar(
                        xn, y_i, mv[:, 0:1], rstd,
                        op0=mybir.AluOpType.subtract, op1=mybir.AluOpType.mult)
                    nc.gpsimd.tensor_mul(out_tile, xn, g_s)
                    nc.gpsimd.tensor_add(out_tile, out_tile, c_s)

            def load_inputs(blk):
                """DMA-transpose-load block inputs (dT first: scores need
                it). dT is split into k-halves so consumers of early
                k-tiles need not wait for the full transfer."""
                r0 = blk * r_blk
                dTa = fm.tile([128, KT // 2, N], BF16, tag="dTa", bufs=2)
                dTb = fm.tile([128, KT // 2, N], BF16, tag="dTb", bufs=2)
                xtT = fm.tile([128, KT, N], BF16, tag="xtT", bufs=2)
                for j in range(KT):
                    h = (dTa, dTb)[j // 4]
                    nc.sync.dma_start(h[:, j % 4, :],
                                      db_d[r0:r0 + N, 128 * j:128 * (j + 1)],
                                      transpose=True)
                for j in range(KT):
                    nc.sync.dma_start(xtT[:, j, :],
                                      xtb_d[r0:r0 + N, 128 * j:128 * (j + 1)],
                                      transpose=True)
                return xtT, (dTa, dTb)

            def dk(dT, k):
                return dT[k // 4][:, k % 4, :]


            def scores_mm(dT):
                """Raw attention scores A @ d -> PSUM (sigmoid deferred)."""
                psc = ps_misc.tile([H, N], F32, tag="misc")
                for k in range(KT):
                    nc.tensor.matmul(psc, at_s[:, k, :], dk(dT, k),
                                     start=(k == 0), stop=(k == KT - 1))
                return psc

            def scores_act(psc):
                attn0 = st.tile([H, N], BF16, tag="attn0", bufs=2)
                nc.scalar.activation(attn0, psc, AF.Sigmoid)
                return attn0

            def attn_u(dT, m):
                pu = ps_mm.tile([128, N], F32, tag="mm", name="pu")
                for k in range(KT):
                    nc.tensor.matmul(pu,
                                     wv_s[:, k, 128 * m:128 * (m + 1)],
                                     dk(dT, k),
                                     start=(k == 0), stop=(k == KT - 1))
                return pu

            def attn_t(t_tile, dT, attn0, m, pu=None):
                if pu is None:
                    pu = attn_u(dT, m)
                pab = ps_ab.tile([128, N], F32, name="pab")
                nc.tensor.matmul(pab, e_s[:, m, :], attn0,
                                 start=True, stop=True)
                # the BIR verifier rejects TensorTensor with two PSUM
                # operands: drain the broadcast through ACT first
                ab_s = work.tile([128, N], BF16, tag="ab", bufs=2)
                nc.scalar.copy(ab_s, pab)
                nc.vector.tensor_mul(t_tile[:, m, :], pu, ab_s)

            # ---- cold start: block 0 inputs + resident weights, ordered
            # by first use (scores->dT, u->wv, y->xtT/wm/wo) --------------
            dT0a = fm.tile([128, KT // 2, N], BF16, tag="dTa", bufs=2)
            dT0b = fm.tile([128, KT // 2, N], BF16, tag="dTb", bufs=2)
            dT0 = (dT0a, dT0b)
            xtT0 = fm.tile([128, KT, N], BF16, tag="xtT", bufs=2)
            wv_s = wres.tile([128, KT, D], BF16)
            wm_s = wres.tile([128, KT, D], BF16)
            wo_s = wres.tile([128, KT, D], BF16)
            for j in range(KT):
                h = (dT0a, dT0b)[j // 4]
                nc.sync.dma_start(h[:, j % 4, :],
                                  db_d[0:N, 128 * j:128 * (j + 1)],
                                  transpose=True)
            for mh in range(2):
                nc.sync.dma_start(wv_s[:, :, 512 * mh:512 * (mh + 1)],
                                  wv_d[:, :, 512 * mh:512 * (mh + 1)])
            for j in range(KT):
                nc.sync.dma_start(xtT0[:, j, :],
                                  xtb_d[0:N, 128 * j:128 * (j + 1)],
                                  transpose=True)
            for q in range(4):
                for w_s, w_d_ in ((wm_s, wm_d), (wo_s, wo_d)):
                    nc.sync.dma_start(w_s[:, :, 256 * q:256 * (q + 1)],
                                      w_d_[:, :, 256 * q:256 * (q + 1)])
            nxt = (xtT0, dT0)
            nxt_at = scores_act(scores_mm(dT0))
            t_cur = am.tile([128, MT, N], BF16, tag="t", bufs=2, name="t_cur")
            pending_ln2 = None

            def emit_ln2(z_rm, r0, tail):
                """Row-major LN2 + store for one block's z_rm tiles.
                tail=True phase-batches the chains (shorter critical path
                at the very end of the kernel); otherwise per-i chains
                with the normalize on Pool (DVE stays free for the
                current block's attention)."""
                g_s = None if trivial_ln2 else g2_s
                c_s = None if trivial_ln2 else c2_s
                if not tail:
                    for i in range(ni):
                        layernorm_rm(z_rm[i], z_rm[i], g_s, c_s)
                        nc.gpsimd.dma_start(
                            out_d[r0 + 128 * i:r0 + 128 * (i + 1), :], z_rm[i])
                    return
                stts, mvs, stds, rstds = [], [], [], []
                for i in range(ni):
                    stt = st.tile([128, 2, 6], F32, tag="bnstT", bufs=ni,
                                  name="stt")
                    nc.vector.bn_stats(stt[:, 0, :], z_rm[i][:, 0:512])
                    nc.vector.bn_stats(stt[:, 1, :], z_rm[i][:, 512:1024])
                    stts.append(stt)
                for i in range(ni):
                    mv = st.tile([128, 2], F32, tag="bnmvT", bufs=ni, name="mv")
                    nc.vector.bn_aggr(mv, stts[i])
                    mvs.append(mv)
                for i in range(ni):
                    std = st.tile([128, 1], F32, tag="bnstdT", bufs=ni,
                                  name="std_i")
                    nc.scalar.activation(std, mvs[i][:, 1:2], AF.Sqrt,
                                         bias=eps_s)
                    stds.append(std)
                for i in range(ni):
                    rstd = st.tile([128, 1], F32, tag="bnrstdT", bufs=ni,
                                   name="rstd_i")
                    nc.vector.reciprocal(rstd, stds[i])
                    rstds.append(rstd)
                for i in range(ni):
                    nc.vector.tensor_scalar(
                        z_rm[i], z_rm[i], mvs[i][:, 0:1], rstds[i],
                        op0=mybir.AluOpType.subtract,
                        op1=mybir.AluOpType.mult)
                    if g_s is not None:
                        nc.vector.tensor_mul(z_rm[i], z_rm[i], g_s)
                        nc.vector.tensor_add(z_rm[i], z_rm[i], c_s)
                    # alternate queues so the final stores overlap; SP is
                    # idle here (no more weight traffic)
                    q = nc.sync if i % 2 == 0 else nc.gpsimd
                    q.dma_start(
                        out_d[r0 + 128 * i:r0 + 128 * (i + 1), :], z_rm[i])

            for blk in range(nb):
                r0 = blk * r_blk
                xtT, dT = nxt
                attn0 = nxt_at

                # next block's inputs: the sync DMA queue is empty here
                if blk + 1 < nb:
                    nxt = load_inputs(blk + 1)

                # ---- attention: t = attn0 * (w_v @ d) -------------------
                # (m=0,1 of this block were pulled into the previous
                # block's LN1 window as PE filler)
                for m in range(0 if blk == 0 else 2, MT):
                    attn_t(t_cur, dT, attn0, m)

                # previous block's LN2 + store, emitted here so its DVE
                # chains fill this block's y-group window instead of
                # racing this block's t-mults
                if pending_ln2 is not None:
                    emit_ln2(*pending_ln2, tail=False)
                    pending_ln2 = None

                # ---- y' = (C w_o w_v)@xt + (C w_o)@t + bodC  (centered) -
                yp = am.tile([128, MT, N], BF16, tag="yp")
                ss = ps_misc.tile([1, N], F32, tag="misc")
                for m in range(MT):
                    py = ps_mm.tile([128, N], F32, tag="mm")
                    for k in range(KT):
                        nc.tensor.matmul(py,
                                         wm_s[:, k, 128 * m:128 * (m + 1)],
                                         xtT[:, k, :],
                                         start=(k == 0), stop=False)
                    for k in range(KT):
                        nc.tensor.matmul(py,
                                         wo_s[:, k, 128 * m:128 * (m + 1)],
                                         t_cur[:, k, :],
                                         start=False, stop=(k == KT - 1))
                    nc.scalar.activation(yp[:, m, :], py, AF.Identity,
                                         bias=bod_s[:, m:m + 1])
                    y2 = work.tile([128, N], BF16, tag="y2")
                    nc.scalar.activation(y2, py, AF.Square,
                                         bias=bod_s[:, m:m + 1])
                    nc.tensor.matmul(ss, ones_col, y2,
                                     start=(m == 0), stop=(m == MT - 1))

                # prefetch the first two FFN1 weight slices now
                w1_pre = []
                for mg in range(2):
                    w1_s = w1pool.tile([128, KT, 512], BF16, tag="wA",
                                       name="w1_s")
                    nc.sync.dma_start(
                        w1_s,
                        w1_d[:, mg // 2, :, 512 * (mg % 2):512 * (mg % 2 + 1)])
                    w1_pre.append(w1_s)

                # ---- LN1 scale: x = y' * rsqrt(mean(y'^2)+eps) ----------
                # The sqrt->recip->bcast chain has no PE work of its own;
                # next block's scores + attention m=0,1 fill the bubble.
                std = st.tile([1, N], F32, tag="std")
                nc.scalar.activation(std, ss, AF.Sqrt, bias=eps1, scale=1.0 / D)
                rstd_bf = st.tile([1, N], BF16, tag="rstdb")
                with nc.allow_low_precision(
                        reason="bf16 rstd: 0.1%% scale noise, LN2 renormalizes"):
                    nc.vector.reciprocal(rstd_bf, std)
                if blk + 1 < nb:
                    nxt_at = scores_act(scores_mm(nxt[1]))
                    t_nxt = am.tile([128, MT, N], BF16, tag="t", bufs=2,
                                    name="t_nxt")
                    pu0 = attn_u(nxt[1], 0)
                    pu1 = attn_u(nxt[1], 1)
                    attn_t(t_nxt, nxt[1], nxt_at, 0, pu=pu0)
                    attn_t(t_nxt, nxt[1], nxt_at, 1, pu=pu1)
                pr1 = ps_misc.tile([128, N], F32, tag="misc")
                nc.tensor.matmul(pr1, ones_row, rstd_bf, start=True, stop=True)
                x_s = am.tile([128, MT, N], BF16, tag="x")
                for m in range(MT):
                    if trivial_ln1:
                        nc.vector.tensor_mul(x_s[:, m, :], pr1, yp[:, m, :])
                    else:
                        xm = work.tile([128, N], F32, tag="xm")
                        nc.vector.tensor_mul(xm, pr1, yp[:, m, :])
                        nc.vector.tensor_scalar(
                            x_s[:, m, :], xm, g1_s[:, m:m + 1], c1_s[:, m:m + 1],
                            op0=mybir.AluOpType.mult, op1=mybir.AluOpType.add)

                # ---- FFN1: h = gelu(w1 @ x + b1) ------------------------
                hT = am.tile([128, FMT, N], BF16, tag="hT")
                for mg in range(8):
                    if mg < 2:
                        w1_s = w1_pre[mg]
                    else:
                        w1_s = w1pool.tile([128, KT, 512], BF16, tag="wA",
                                           name="w1_s")
                        nc.sync.dma_start(
                            w1_s,
                            w1_d[:, mg // 2, :,
                                 512 * (mg % 2):512 * (mg % 2 + 1)])
                    for mm in range(4):
                        m = mg * 4 + mm
                        pm = ps_mm.tile([128, N], F32, tag="mm")
                        for k in range(KT):
                            nc.tensor.matmul(pm,
                                             w1_s[:, k, 128 * mm:128 * (mm + 1)],
                                             x_s[:, k, :],
                                             start=(k == 0), stop=(k == KT - 1))
                        nc.scalar.activation(hT[:, m, :], pm, AF.Gelu,
                                             bias=b1_s[:, m:m + 1])

                # ---- FFN2 + residual + bf16 transpose -------------------
                # w2 stream runs one half-slice ahead of the consuming
                # matmuls (bufs=2: one in use, one loading).
                def load_w2(m, kh):
                    w2_s = w2pool.tile([128, 16, 128], BF16, tag="w2s",
                                       name="w2_s")
                    nc.sync.dma_start(w2_s,
                                      w2_d[:, m, 16 * kh:16 * (kh + 1), :])
                    return w2_s

                w2_nxt = load_w2(0, 0)
                z_rm = [rm.tile([128, D], F32, tag="z_rm", bufs=ni,
                                name="z_rm")
                        for _ in range(ni)]
                for m in range(MT):
                    pm = ps_mm.tile([128, N], F32, tag="mm")
                    for kh in range(2):
                        w2_s = w2_nxt
                        if not (m == MT - 1 and kh == 1):
                            w2_nxt = load_w2(m + kh, (kh + 1) % 2)
                        for kk in range(16):
                            k = 16 * kh + kk
                            nc.tensor.matmul(pm, w2_s[:, kk, :], hT[:, k, :],
                                             start=(k == 0),
                                             stop=(k == FMT - 1))
                    fz = work.tile([128, N], BF16, tag="fz")
                    nc.scalar.activation(fz, pm, AF.Identity,
                                         bias=b2_s[:, m:m + 1])
                    zt = work.tile([128, N], BF16, tag="zt")
                    nc.vector.tensor_add(zt, x_s[:, m, :], fz)
                    ptr = ps_tr.tile([128, ni * 128], BF16)
                    for i in range(ni):
                        nc.tensor.transpose(ptr[:, 128 * i:128 * (i + 1)],
                                            zt[:, 128 * i:128 * (i + 1)], ident)
                    for i in range(ni):
                        nc.scalar.copy(z_rm[i][:, 128 * m:128 * (m + 1)],
                                       ptr[:, 128 * i:128 * (i + 1)])

                pending_ln2 = (z_rm, r0)
                if blk + 1 < nb:
                    t_cur = t_nxt
            emit_ln2(*pending_ln2, tail=True)

    nc.compile()
    return nc


def host_prepare(inputs):
    """Fold parameters and lay out weights for the device (all O(params))."""
    f64 = {k: np.asarray(inputs[k], dtype=np.float64)
           for k in ("dom_movie", "w_q", "w_k", "w_v", "b_q", "w_o", "b_o",
                     "b_v")}
    qs = (f64["dom_movie"] @ f64["w_q"].T + f64["b_q"]) / np.sqrt(HD)  # (1, D)
    qh = qs.reshape(H, HD)
    A = np.einsum("hd,hdD->hD", qh, f64["w_k"].reshape(H, HD, D))  # (H, D)
    bod2 = f64["b_o"] + f64["dom_movie"][0] + f64["w_o"] @ f64["b_v"]  # (D,)

    # LN centering folded into the output projection: C = I - 11^T/D
    woC = f64["w_o"] - f64["w_o"].mean(axis=0, keepdims=True)   # C @ w_o
    M = woC @ f64["w_v"]                                        # C w_o w_v
    bodC = bod2 - bod2.mean()                                   # C @ bod2

    E = np.zeros((H, MT, 128), np.float32)
    for m in range(MT):
        for p in range(128):
            E[2 * m + p // 64, m, p] = 1.0

    w1 = np.asarray(inputs["w1"], np.float32)
    w2 = np.asarray(inputs["w2"], np.float32)

    def fm_weight(wT):  # wT (d_in, d_out) -> [128, d_in/128, d_out]
        return np.ascontiguousarray(
            wT.reshape(-1, 128, wT.shape[1]).transpose(1, 0, 2)).astype(NPBF16)

    prep = {
        "wvT": fm_weight(np.asarray(f64["w_v"], np.float32).T),
        "wmT": fm_weight(np.asarray(M, np.float32).T),
        "woT": fm_weight(np.asarray(woC, np.float32).T),
        "w1P": np.ascontiguousarray(
            w1.T.reshape(KT, 128, NMG, D).transpose(1, 2, 0, 3)).astype(NPBF16),
        "w2P": np.ascontiguousarray(
            w2.T.reshape(FMT, 128, MT, 128).transpose(1, 2, 0, 3)).astype(NPBF16),
        "AT": np.ascontiguousarray(
            A.T.reshape(KT, 128, H).transpose(1, 0, 2)).astype(NPBF16),
        "E": E.astype(NPBF16),
        "bodC": np.ascontiguousarray(
            bodC.reshape(MT, 128).T).astype(np.float32),
        "b1p": np.ascontiguousarray(
            np.asarray(inputs["b1"], np.float64).reshape(FMT, 128).T
        ).astype(np.float32),
        "b2p": np.ascontiguousarray(
            np.asarray(inputs["b2"], np.float64).reshape(MT, 128).T
        ).astype(np.float32),
    }
    trivial_ln1 = bool(np.all(np.asarray(inputs["ln1_g"]) == 1.0)
                       and np.all(np.asarray(inputs["ln1_b"]) == 0.0))
    trivial_ln2 = bool(np.all(np.asarray(inputs["ln2_g"]) == 1.0)
                       and np.all(np.asarray(inputs["ln2_b"]) == 0.0))
    if not trivial_ln1:
        prep["g1p"] = np.ascontiguousarray(
            np.asarray(inputs["ln1_g"], np.float64).reshape(MT, 128).T
        ).astype(np.float32)
        prep["c1p"] = np.ascontiguousarray(
            np.asarray(inputs["ln1_b"], np.float64).reshape(MT, 128).T
        ).astype(np.float32)
    if not trivial_ln2:
        prep["g2"] = np.asarray(inputs["ln2_g"], np.float32)
        prep["c2"] = np.asarray(inputs["ln2_b"], np.float32)
    return prep, trivial_ln1, trivial_ln2


_PROGRAM_CACHE = {}


def _get_program(b_core, r_blk, t1, t2):
    key = (b_core, r_blk, t1, t2)
    if key not in _PROGRAM_CACHE:
        _PROGRAM_CACHE[key] = build_program(b_core, r_blk, t1, t2)
    return _PROGRAM_CACHE[key]


def kernel(h_u_cross, h_u_target, dom_movie, w_q, w_k, w_v, b_q, b_k, b_v,
           w_o, b_o, ln1_g, ln1_b, w1, b1, w2, b2, ln2_g, ln2_b,
           trace=False, r_blk=512, **run_kwargs):
    inputs = dict(h_u_cross=h_u_cross, h_u_target=h_u_target,
                  dom_movie=dom_movie, w_q=w_q, w_k=w_k, w_v=w_v, b_q=b_q,
                  b_k=b_k, b_v=b_v, w_o=w_o, b_o=b_o, ln1_g=ln1_g,
                  ln1_b=ln1_b, w1=w1, b1=b1, w2=w2, b2=b2, ln2_g=ln2_g,
                  ln2_b=ln2_b)
    prep, t1, t2 = host_prepare(inputs)
    nc = _get_program(B_CORE, r_blk, t1, t2)

    xc = np.asarray(h_u_cross, np.float32)
    xt = np.asarray(h_u_target, np.float32)
    xtb = np.ascontiguousarray(xt.astype(NPBF16))
    db = np.ascontiguousarray((xc - xt).astype(NPBF16))
    in_maps = []
    for c in range(N_CORES):
        m = dict(prep)
        m["xtb"] = xtb[c * B_CORE:(c + 1) * B_CORE]
        m["db"] = db[c * B_CORE:(c + 1) * B_CORE]
        in_maps.append(m)

    res = run_bass_kernel_spmd(nc, in_maps, core_ids=list(range(N_CORES)),
                               trace=trace, **run_kwargs)
    out = np.concatenate([res.results[c]["out"] for c in range(N_CORES)], axis=0)
    kernel.last_results = res
    return out.astype(np.float32)


# revision 28
# speedup vs baseline: 1.0081x; 1.0081x over previous
"""Trainium2 Bass kernel for AttentionConditionGenerator.

Reference computation (per row b of B=16384):
    kv = [h_u_cross[b], h_u_target[b]]            # (2, 1024)
    q  = dom_movie @ w_q.T + b_q                  # fixed across rows
    scores = (q/8) . k[s],  attn = softmax_2(scores)
    ctx = attn0*v0 + attn1*v1 ; y = ctx @ w_o.T + b_o
    x = LN1(dom_movie + y); h = gelu(x @ w1.T + b1)
    out = LN2(x + h @ w2.T + b2)

Algebraic folding (host, fp64, exact):
  - q row-independent -> scores fold to A @ d with d = xc - xt;
    attn0 = sigmoid(A @ d) (b_k cancels in the 2-way softmax).
  - ctx = v_t + attn0 * v_d, v_t = w_v@xt + b_v, v_d = w_v@d.
  - LN1 centering folded into the weights: with C = I - 11^T/D,
        y' = C@y = (C@w_o@w_v)@xt + (C@w_o)@(attn0*(w_v@d)) + C@bod2
    so y' arrives centered and LN1 reduces to a per-row scale:
        x = y' * rsqrt(mean(y'^2) + eps)
    x is exactly column-centered too, so LN2 sees a centered residual.

Device mapping: batch split over 8 cores (2048 rows each). Activations stay
feature-major (features on partitions) end-to-end; LN1 runs feature-major
(variance via ones-vector matmuls + a 1-row broadcast matmul), LN2 runs
row-major after cheap bf16 transposes of the final residual. All matmuls are
bf16 with fp32 PSUM accumulation.
"""

import numpy as np
import ml_dtypes

try:
    import concourse.bass as bass
except ImportError:  # pragma: no cover - path setup for fresh environments
    import sys

    for _p in ("/opt/trn_rl_repo", "/root/.axon_site/_ro/trn_rl_repo"):
        if _p not in sys.path:
            sys.path.insert(0, _p)
    import concourse.bass as bass

import concourse.mybir as mybir
import concourse.tile as tile
from concourse import bacc
from concourse.bass_utils import run_bass_kernel_spmd
from concourse.masks import make_identity

F32 = mybir.dt.float32
BF16 = mybir.dt.bfloat16
NPBF16 = ml_dtypes.bfloat16

D = 1024
H = 16
HD = 64
FFN = 4096
EPS = 1e-5
N_CORES = 8
B_TOTAL = 16384
B_CORE = B_TOTAL // N_CORES  # 2048

KT = D // 128  # 8 feature k-tiles
MT = D // 128  # 8 output m-tiles
FMT = FFN // 128  # 32 FFN m-tiles
NMG = 4  # host-side FFN1 m-group axis (1024 cols each)

AF = mybir.ActivationFunctionType


def build_program(b_core, r_blk, trivial_ln1, trivial_ln2):
    """Build and compile the per-core Bass program."""
    nb = b_core // r_blk  # row blocks
    ni = r_blk // 128  # 128-row subtiles per block
    N = r_blk  # matmul moving (free) dim

    nc = bacc.Bacc("TRN2", target_bir_lowering=False)

    # ---- DRAM I/O ------------------------------------------------------
    xtb_d = nc.dram_tensor("xtb", [b_core, D], BF16, kind="ExternalInput")
    db_d = nc.dram_tensor("db", [b_core, D], BF16, kind="ExternalInput")
    wv_d = nc.dram_tensor("wvT", [128, KT, D], BF16, kind="ExternalInput")
    wm_d = nc.dram_tensor("wmT", [128, KT, D], BF16, kind="ExternalInput")
    wo_d = nc.dram_tensor("woT", [128, KT, D], BF16, kind="ExternalInput")
    w1_d = nc.dram_tensor("w1P", [128, NMG, KT, D], BF16, kind="ExternalInput")
    w2_d = nc.dram_tensor("w2P", [128, MT, FMT, 128], BF16, kind="ExternalInput")
    at_d = nc.dram_tensor("AT", [128, KT, H], BF16, kind="ExternalInput")
    e_d = nc.dram_tensor("E", [H, MT, 128], BF16, kind="ExternalInput")
    bod_d = nc.dram_tensor("bodC", [128, MT], F32, kind="ExternalInput")
    b1_d = nc.dram_tensor("b1p", [128, FMT], F32, kind="ExternalInput")
    b2_d = nc.dram_tensor("b2p", [128, MT], F32, kind="ExternalInput")
    if not trivial_ln1:
        g1_d = nc.dram_tensor("g1p", [128, MT], F32, kind="ExternalInput")
        c1_d = nc.dram_tensor("c1p", [128, MT], F32, kind="ExternalInput")
    if not trivial_ln2:
        g2_d = nc.dram_tensor("g2", [D], F32, kind="ExternalInput")
        c2_d = nc.dram_tensor("c2", [D], F32, kind="ExternalInput")
    out_d = nc.dram_tensor("out", [b_core, D], F32, kind="ExternalOutput")

    def bcast_ap(dram):
        # [D] dram vector -> [128, D] partition-broadcast access pattern
        return bass.AP(tensor=dram, offset=0, ap=[[0, 128], [1, D]])

    with tile.TileContext(nc) as tc:
        with (
            tc.tile_pool(name="consts", bufs=1) as consts,
            tc.tile_pool(name="wres", bufs=1) as wres,
            tc.tile_pool(name="w1s", bufs=2) as w1pool,
            tc.tile_pool(name="w2s", bufs=2) as w2pool,
            tc.tile_pool(name="fm", bufs=1) as fm,
            tc.tile_pool(name="act", bufs=1) as am,
            tc.tile_pool(name="work", bufs=3) as work,
            tc.tile_pool(name="stats", bufs=2) as st,
            tc.tile_pool(name="rm", bufs=2) as rm,
            tc.tile_pool(name="ps_mm", bufs=3, space="PSUM") as ps_mm,
            tc.tile_pool(name="ps_ab", bufs=2, space="PSUM") as ps_ab,
            tc.tile_pool(name="ps_misc", bufs=2, space="PSUM") as ps_misc,
            tc.tile_pool(name="ps_tr", bufs=1, space="PSUM") as ps_tr,
        ):
            # ---- constants (at_s first: scores need it immediately;
            # the rest are loaded after the cold input transposes) -----
            at_s = consts.tile([128, KT, H], BF16)
            nc.sync.dma_start(at_s, at_d[:, :, :])
            ident = consts.tile([128, 128], BF16)
            e_s = consts.tile([H, MT, 128], BF16)
            bod_s = consts.tile([128, MT], F32)
            b1_s = consts.tile([128, FMT], F32)
            b2_s = consts.tile([128, MT], F32)
            ones_col = consts.tile([128, 1], BF16)
            nc.vector.memset(ones_col, 1.0)
            ones_row = consts.tile([1, 128], BF16)
            nc.vector.memset(ones_row, 1.0)
            eps_s = consts.tile([128, 1], F32)
            nc.vector.memset(eps_s, EPS)
            eps1 = consts.tile([1, 1], F32)
            nc.vector.memset(eps1, EPS)
            g1_s = c1_s = g2_s = c2_s = None
            if not trivial_ln1:
                g1_s = consts.tile([128, MT], F32)
                nc.sync.dma_start(g1_s, g1_d[:, :])
                c1_s = consts.tile([128, MT], F32)
                nc.sync.dma_start(c1_s, c1_d[:, :])
            if not trivial_ln2:
                g2_s = consts.tile([128, D], F32)
                nc.gpsimd.dma_start(g2_s, bcast_ap(g2_d))
                c2_s = consts.tile([128, D], F32)
                nc.gpsimd.dma_start(c2_s, bcast_ap(c2_d))

            def layernorm_rm(y_i, out_tile, g_s, c_s, last=False):
                """Row-major LayerNorm of y_i [128, D] f32 -> out_tile.
                last=True puts the normalize on DVE (the Pool impl is
                ~2x slower, which only matters on the kernel tail)."""
                stt = st.tile([128, 2, 6], F32, tag="bnst")
                nc.vector.bn_stats(stt[:, 0, :], y_i[:, 0:512])
                nc.vector.bn_stats(stt[:, 1, :], y_i[:, 512:1024])
                mv = st.tile([128, 2], F32, tag="bnmv")
                nc.vector.bn_aggr(mv, stt)
                std = st.tile([128, 1], F32, tag="bnstd")
                nc.scalar.activation(std, mv[:, 1:2], AF.Sqrt, bias=eps_s)
                rstd = st.tile([128, 1], F32, tag="bnrstd")
                nc.vector.reciprocal(rstd, std)
                eng = nc.vector if last else nc.gpsimd
                if g_s is None:
                    eng.tensor_scalar(
                        out_tile, y_i, mv[:, 0:1], rstd,
                        op0=mybir.AluOpType.subtract, op1=mybir.AluOpType.mult)
                else:
                    xn = st.tile([128, D], F32, tag="bnxn")
                    nc.gpsimd.tensor_scalar(
                        xn, y_i, mv[:, 0:1], rstd,
                        op0=mybir.AluOpType.subtract, op1=mybir.AluOpType.mult)
                    nc.gpsimd.tensor_mul(out_tile, xn, g_s)
                    nc.gpsimd.tensor_add(out_tile, out_tile, c_s)

            def load_inputs(blk):
                """DMA-transpose-load block inputs (dT first: scores need
                it). dT is split into k-halves so consumers of early
                k-tiles need not wait for the full transfer."""
                r0 = blk * r_blk
                dTa = fm.tile([128, KT // 2, N], BF16, tag="dTa", bufs=2)
                dTb = fm.tile([128, KT // 2, N], BF16, tag="dTb", bufs=2)
                xtT = fm.tile([128, KT, N], BF16, tag="xtT", bufs=2)
                for j in range(KT):
                    h = (dTa, dTb)[j // 4]
                    nc.sync.dma_start(h[:, j % 4, :],
                                      db_d[r0:r0 + N, 128 * j:128 * (j + 1)],
                                      transpose=True)
                for j in range(KT):
                    nc.sync.dma_start(xtT[:, j, :],
                                      xtb_d[r0:r0 + N, 128 * j:128 * (j + 1)],
                                      transpose=True)
                return xtT, (dTa, dTb)

            def dk(dT, k):
                return dT[k // 4][:, k % 4, :]


            def scores_mm(dT):
                """Raw attention scores A @ d -> PSUM (sigmoid deferred)."""
                psc = ps_misc.tile([H, N], F32, tag="misc")
                for k in range(KT):
                    nc.tensor.matmul(psc, at_s[:, k, :], dk(dT, k),
                                     start=(k == 0), stop=(k == KT - 1))
                return psc

            def scores_act(psc):
                attn0 = st.tile([H, N], BF16, tag="attn0", bufs=2)
                nc.scalar.activation(attn0, psc, AF.Sigmoid)
                return attn0

            def attn_u(dT, m):
                pu = ps_mm.tile([128, N], F32, tag="mm", name="pu")
                for k in range(KT):
                    nc.tensor.matmul(pu,
                                     wv_s[:, k, 128 * m:128 * (m + 1)],
                                     dk(dT, k),
                                     start=(k == 0), stop=(k == KT - 1))
                return pu

            def attn_t(t_tile, dT, attn0, m, pu=None):
                if pu is None:
                    pu = attn_u(dT, m)
                pab = ps_ab.tile([128, N], F32, name="pab")
                nc.tensor.matmul(pab, e_s[:, m, :], attn0,
                                 start=True, stop=True)
                # the BIR verifier rejects TensorTensor with two PSUM
                # operands: drain the broadcast through ACT first
                ab_s = work.tile([128, N], BF16, tag="ab", bufs=2)
                nc.scalar.copy(ab_s, pab)
                nc.vector.tensor_mul(t_tile[:, m, :], pu, ab_s)

            # ---- cold start: block 0 inputs + resident weights, ordered
            # by first use (scores->dT, u->wv, y->xtT/wm/wo) --------------
            dT0a = fm.tile([128, KT // 2, N], BF16, tag="dTa", bufs=2)
            dT0b = fm.tile([128, KT // 2, N], BF16, tag="dTb", bufs=2)
            dT0 = (dT0a, dT0b)
            xtT0 = fm.tile([128, KT, N], BF16, tag="xtT", bufs=2)
            wv_s = wres.tile([128, KT, D], BF16)
            wm_s = wres.tile([128, KT, D], BF16)
            wo_s = wres.tile([128, KT, D], BF16)
            # first input: dT halves split across the two DMA queues so
            # HWDGE descriptor dispatch (625ns each) isn't the critical
            # path into the first score/u matmuls
            for j in range(4):
                nc.sync.dma_start(dT0a[:, j, :],
                                  db_d[0:N, 128 * j:128 * (j + 1)],
                                  transpose=True)
                nc.scalar.dma_start(dT0b[:, j, :],
                                    db_d[0:N, 128 * (j + 4):128 * (j + 5)],
                                    transpose=True)
            for mh in range(2):
                nc.sync.dma_start(wv_s[:, :, 512 * mh:512 * (mh + 1)],
                                  wv_d[:, :, 512 * mh:512 * (mh + 1)])
            for j in range(4):
                nc.scalar.dma_start(xtT0[:, j, :],
                                    xtb_d[0:N, 128 * j:128 * (j + 1)],
                                    transpose=True)
            make_identity(nc, ident)
            nc.gpsimd.dma_start(e_s, e_d[:, :, :])
            for j in range(4, KT):
                nc.sync.dma_start(xtT0[:, j, :],
                                  xtb_d[0:N, 128 * j:128 * (j + 1)],
                                  transpose=True)
            nc.sync.dma_start(bod_s, bod_d[:, :])
            for q in range(4):
                for w_s, w_d_ in ((wm_s, wm_d), (wo_s, wo_d)):
                    nc.sync.dma_start(w_s[:, :, 256 * q:256 * (q + 1)],
                                      w_d_[:, :, 256 * q:256 * (q + 1)])
            nc.sync.dma_start(b1_s, b1_d[:, :])
            nc.sync.dma_start(b2_s, b2_d[:, :])
            nxt = (xtT0, dT0)
            nxt_at = scores_act(scores_mm(dT0))
            t_cur = am.tile([128, MT, N], BF16, tag="t", bufs=2, name="t_cur")
            pending_ln2 = None

            def emit_ln2(z_rm, r0, tail):
                """Row-major LN2 + store for one block's z_rm tiles.
                tail=True phase-batches the chains (shorter critical path
                at the very end of the kernel); otherwise per-i chains
                with the normalize on Pool (DVE stays free for the
                current block's attention)."""
                g_s = None if trivial_ln2 else g2_s
                c_s = None if trivial_ln2 else c2_s
                if not tail:
                    for i in range(ni):
                        layernorm_rm(z_rm[i], z_rm[i], g_s, c_s)
                        nc.gpsimd.dma_start(
                            out_d[r0 + 128 * i:r0 + 128 * (i + 1), :], z_rm[i])
                    return
                stts, mvs, stds, rstds = [], [], [], []
                for i in range(ni):
                    stt = st.tile([128, 2, 6], F32, tag="bnstT", bufs=ni,
                                  name="stt")
                    nc.vector.bn_stats(stt[:, 0, :], z_rm[i][:, 0:512])
                    nc.vector.bn_stats(stt[:, 1, :], z_rm[i][:, 512:1024])
                    stts.append(stt)
                for i in range(ni):
                    mv = st.tile([128, 2], F32, tag="bnmvT", bufs=ni, name="mv")
                    nc.vector.bn_aggr(mv, stts[i])
                    mvs.append(mv)
                for i in range(ni):
                    std = st.tile([128, 1], F32, tag="bnstdT", bufs=ni,
                                  name="std_i")
                    nc.scalar.activation(std, mvs[i][:, 1:2], AF.Sqrt,
                                         bias=eps_s)
                    stds.append(std)
                for i in range(ni):
                    rstd = st.tile([128, 1], F32, tag="bnrstdT", bufs=ni,
                                   name="rstd_i")
                    nc.vector.reciprocal(rstd, stds[i])
                    rstds.append(rstd)
                for i in range(ni):
                    nc.vector.tensor_scalar(
                        z_rm[i], z_rm[i], mvs[i][:, 0:1], rstds[i],
                        op0=mybir.AluOpType.subtract,
                        op1=mybir.AluOpType.mult)
                    if g_s is not None:
                        nc.vector.tensor_mul(z_rm[i], z_rm[i], g_s)
                        nc.vector.tensor_add(z_rm[i], z_rm[i], c_s)
                    # alternate queues so the final stores overlap; SP is
                    # idle here (no more weight traffic)
                    q = nc.sync if i % 2 == 0 else nc.gpsimd
                    q.dma_start(
                        out_d[r0 + 128 * i:r0 + 128 * (i + 1), :], z_rm[i])

            for blk in range(nb):
                r0 = blk * r_blk
                xtT, dT = nxt
                attn0 = nxt_at

                # next block's inputs: the sync DMA queue is empty here
                if blk + 1 < nb:
                    nxt = load_inputs(blk + 1)

                # ---- attention: t = attn0 * (w_v @ d) -------------------
                # (m=0,1 of this block were pulled into the previous
                # block's LN1 window as PE filler)
                for m in range(0 if blk == 0 else 2, MT):
                    attn_t(t_cur, dT, attn0, m)

                # previous block's LN2 + store, emitted here so its DVE
                # chains fill this block's y-group window instead of
                # racing this block's t-mults
                if pending_ln2 is not None:
                    emit_ln2(*pending_ln2, tail=False)
                    pending_ln2 = None

                # ---- y' = (C w_o w_v)@xt + (C w_o)@t + bodC  (centered) -
                yp = am.tile([128, MT, N], BF16, tag="yp")
                ss = ps_misc.tile([1, N], F32, tag="misc")
                for m in range(MT):
                    py = ps_mm.tile([128, N], F32, tag="mm")
                    for k in range(KT):
                        nc.tensor.matmul(py,
                                         wm_s[:, k, 128 * m:128 * (m + 1)],
                                         xtT[:, k, :],
                                         start=(k == 0), stop=False)
                    for k in range(KT):
                        nc.tensor.matmul(py,
                                         wo_s[:, k, 128 * m:128 * (m + 1)],
                                         t_cur[:, k, :],
                                         start=False, stop=(k == KT - 1))
                    nc.scalar.activation(yp[:, m, :], py, AF.Identity,
                                         bias=bod_s[:, m:m + 1])
                    y2 = work.tile([128, N], BF16, tag="y2", bufs=2)
                    nc.scalar.activation(y2, py, AF.Square,
                                         bias=bod_s[:, m:m + 1])
                    nc.tensor.matmul(ss, ones_col, y2,
                                     start=(m == 0), stop=(m == MT - 1))

                # prefetch the first two FFN1 weight slices now
                w1_pre = []
                for mg in range(2):
                    w1_s = w1pool.tile([128, KT, 512], BF16, tag="wA",
                                       name="w1_s")
                    nc.sync.dma_start(
                        w1_s,
                        w1_d[:, mg // 2, :, 512 * (mg % 2):512 * (mg % 2 + 1)])
                    w1_pre.append(w1_s)

                # ---- LN1 scale: x = y' * rsqrt(mean(y'^2)+eps) ----------
                # The sqrt->recip->bcast chain has no PE work of its own;
                # next block's scores + attention m=0,1 fill the bubble.
                std = st.tile([1, N], F32, tag="std")
                nc.scalar.activation(std, ss, AF.Sqrt, bias=eps1, scale=1.0 / D)
                rstd_bf = st.tile([1, N], BF16, tag="rstdb")
                with nc.allow_low_precision(
                        reason="bf16 rstd: 0.1%% scale noise, LN2 renormalizes"):
                    nc.vector.reciprocal(rstd_bf, std)
                if blk + 1 < nb:
                    nxt_at = scores_act(scores_mm(nxt[1]))
                    t_nxt = am.tile([128, MT, N], BF16, tag="t", bufs=2,
                                    name="t_nxt")
                    pu0 = attn_u(nxt[1], 0)
                    pu1 = attn_u(nxt[1], 1)
                    attn_t(t_nxt, nxt[1], nxt_at, 0, pu=pu0)
                    attn_t(t_nxt, nxt[1], nxt_at, 1, pu=pu1)
                pr1 = ps_misc.tile([128, N], F32, tag="misc")
                nc.tensor.matmul(pr1, ones_row, rstd_bf, start=True, stop=True)
                r1_s = work.tile([128, N], BF16, tag="r1", bufs=2)
                nc.scalar.copy(r1_s, pr1)
                x_s = am.tile([128, MT, N], BF16, tag="x")

                def x_mults():
                    for m in range(MT):
                        if trivial_ln1:
                            # all-bf16 SBUF operands: 2x DVE mode
                            nc.vector.tensor_mul(x_s[:, m, :], yp[:, m, :],
                                                 r1_s)
                        else:
                            xm = work.tile([128, N], F32, tag="xm")
                            nc.vector.tensor_mul(xm, yp[:, m, :], r1_s)
                            nc.vector.tensor_scalar(
                                x_s[:, m, :], xm, g1_s[:, m:m + 1],
                                c1_s[:, m:m + 1],
                                op0=mybir.AluOpType.mult,
                                op1=mybir.AluOpType.add)

                if blk + 1 < nb or not trivial_ln1:
                    x_mults()

                # ---- FFN1: h = gelu(w1 @ x + b1) ------------------------
                # Last block: no next-block filler exists for the LN1
                # chain, so run the matmuls on raw y' and fold the
                # per-column rstd scale in after the matmul (exact:
                # w1 @ (y'*r) = (w1 @ y') * r). PE never waits on rstd.
                last_blk = blk == nb - 1 and trivial_ln1
                hT = am.tile([128, FMT, N], BF16, tag="hT")
                for mg in range(8):
                    if mg < 2:
                        w1_s = w1_pre[mg]
                    else:
                        w1_s = w1pool.tile([128, KT, 512], BF16, tag="wA",
                                           name="w1_s")
                        nc.sync.dma_start(
                            w1_s,
                            w1_d[:, mg // 2, :,
                                 512 * (mg % 2):512 * (mg % 2 + 1)])
                    for mm in range(4):
                        m = mg * 4 + mm
                        pm = ps_mm.tile([128, N], F32, tag="mm")
                        f1_src = yp if last_blk else x_s
                        for k in range(KT):
                            nc.tensor.matmul(pm,
                                             w1_s[:, k, 128 * mm:128 * (mm + 1)],
                                             f1_src[:, k, :],
                                             start=(k == 0), stop=(k == KT - 1))
                        if last_blk:
                            tg = work.tile([128, N], BF16, tag="fz", bufs=2,
                                           name="tg")
                            nc.vector.tensor_mul(tg, pm, r1_s)
                            nc.scalar.activation(hT[:, m, :], tg, AF.Gelu,
                                                 bias=b1_s[:, m:m + 1])
                        else:
                            nc.scalar.activation(hT[:, m, :], pm, AF.Gelu,
                                                 bias=b1_s[:, m:m + 1])
                if last_blk:
                    # residual x needed only from FFN2 onward
                    x_mults()

                # ---- FFN2 + residual + bf16 transpose -------------------
                # w2 stream runs one half-slice ahead of the consuming
                # matmuls (bufs=2: one in use, one loading).
                def load_w2(m, kh):
                    w2_s = w2pool.tile([128, 16, 128], BF16, tag="w2s",
                                       name="w2_s")
                    nc.sync.dma_start(w2_s,
                                      w2_d[:, m, 16 * kh:16 * (kh + 1), :])
                    return w2_s

                z_rm = [rm.tile([128, D], F32, tag="z_rm", bufs=ni,
                                name="z_rm")
                        for _ in range(ni)]
                w2_nxt = load_w2(0, 0)
                for m in range(MT):
                    pm = ps_mm.tile([128, N], F32, tag="mm")
                    for kh in range(2):
                        w2_s = w2_nxt
                        if not (m == MT - 1 and kh == 1):
                            w2_nxt = load_w2(m + kh, (kh + 1) % 2)
                        for kk in range(16):
                            k = 16 * kh + kk
                            nc.tensor.matmul(pm, w2_s[:, kk, :], hT[:, k, :],
                                             start=(k == 0),
                                             stop=(k == FMT - 1))
                    fz = work.tile([128, N], BF16, tag="fz", bufs=2)
                    nc.scalar.activation(fz, pm, AF.Identity,
                                         bias=b2_s[:, m:m + 1])
                    zt = work.tile([128, N], BF16, tag="zt")
                    nc.vector.tensor_add(zt, x_s[:, m, :], fz)
                    ptr = ps_tr.tile([128, ni * 128], BF16)
                    for i in range(ni):
                        nc.tensor.transpose(ptr[:, 128 * i:128 * (i + 1)],
                                            zt[:, 128 * i:128 * (i + 1)], ident)
                    for i in range(ni):
                        nc.scalar.copy(z_rm[i][:, 128 * m:128 * (m + 1)],
                                       ptr[:, 128 * i:128 * (i + 1)])

                pending_ln2 = (z_rm, r0)
                if blk + 1 < nb:
                    t_cur = t_nxt
            emit_ln2(*pending_ln2, tail=True)

    nc.compile()
    return nc


def host_prepare(inputs):
    """Fold parameters and lay out weights for the device (all O(params))."""
    f64 = {k: np.asarray(inputs[k], dtype=np.float64)
           for k in ("dom_movie", "w_q", "w_k", "w_v", "b_q", "w_o", "b_o",
                     "b_v")}
    qs = (f64["dom_movie"] @ f64["w_q"].T + f64["b_q"]) / np.sqrt(HD)  # (1, D)
    qh = qs.reshape(H, HD)
    A = np.einsum("hd,hdD->hD", qh, f64["w_k"].reshape(H, HD, D))  # (H, D)
    bod2 = f64["b_o"] + f64["dom_movie"][0] + f64["w_o"] @ f64["b_v"]  # (D,)

    # LN centering folded into the output projection: C = I - 11^T/D
    woC = f64["w_o"] - f64["w_o"].mean(axis=0, keepdims=True)   # C @ w_o
    M = woC @ f64["w_v"]                                        # C w_o w_v
    bodC = bod2 - bod2.mean()                                   # C @ bod2

    E = np.zeros((H, MT, 128), np.float32)
    for m in range(MT):
        for p in range(128):
            E[2 * m + p // 64, m, p] = 1.0

    w1 = np.asarray(inputs["w1"], np.float32)
    w2 = np.asarray(inputs["w2"], np.float32)

    def fm_weight(wT):  # wT (d_in, d_out) -> [128, d_in/128, d_out]
        return np.ascontiguousarray(
            wT.reshape(-1, 128, wT.shape[1]).transpose(1, 0, 2)).astype(NPBF16)

    prep = {
        "wvT": fm_weight(np.asarray(f64["w_v"], np.float32).T),
        "wmT": fm_weight(np.asarray(M, np.float32).T),
        "woT": fm_weight(np.asarray(woC, np.float32).T),
        "w1P": np.ascontiguousarray(
            w1.T.reshape(KT, 128, NMG, D).transpose(1, 2, 0, 3)).astype(NPBF16),
        "w2P": np.ascontiguousarray(
            w2.T.reshape(FMT, 128, MT, 128).transpose(1, 2, 0, 3)).astype(NPBF16),
        "AT": np.ascontiguousarray(
            A.T.reshape(KT, 128, H).transpose(1, 0, 2)).astype(NPBF16),
        "E": E.astype(NPBF16),
        "bodC": np.ascontiguousarray(
            bodC.reshape(MT, 128).T).astype(np.float32),
        "b1p": np.ascontiguousarray(
            np.asarray(inputs["b1"], np.float64).reshape(FMT, 128).T
        ).astype(np.float32),
        "b2p": np.ascontiguousarray(
            np.asarray(inputs["b2"], np.float64).reshape(MT, 128).T
        ).astype(np.float32),
    }
    trivial_ln1 = bool(np.all(np.asarray(inputs["ln1_g"]) == 1.0)
                       and np.all(np.asarray(inputs["ln1_b"]) == 0.0))
    trivial_ln2 = bool(np.all(np.asarray(inputs["ln2_g"]) == 1.0)
                       and np.all(np.asarray(inputs["ln2_b"]) == 0.0))
    if not trivial_ln1:
        prep["g1p"] = np.ascontiguousarray(
            np.asarray(inputs["ln1_g"], np.float64).reshape(MT, 128).T
        ).astype(np.float32)
        prep["c1p"] = np.ascontiguousarray(
            np.asarray(inputs["ln1_b"], np.float64).reshape(MT, 128).T
        ).astype(np.float32)
    if not trivial_ln2:
        prep["g2"] = np.asarray(inputs["ln2_g"], np.float32)
        prep["c2"] = np.asarray(inputs["ln2_b"], np.float32)
    return prep, trivial_ln1, trivial_ln2


_PROGRAM_CACHE = {}


def _get_program(b_core, r_blk, t1, t2):
    key = (b_core, r_blk, t1, t2)
    if key not in _PROGRAM_CACHE:
        _PROGRAM_CACHE[key] = build_program(b_core, r_blk, t1, t2)
    return _PROGRAM_CACHE[key]


def kernel(h_u_cross, h_u_target, dom_movie, w_q, w_k, w_v, b_q, b_k, b_v,
           w_o, b_o, ln1_g, ln1_b, w1, b1, w2, b2, ln2_g, ln2_b,
           trace=False, r_blk=512, **run_kwargs):
    inputs = dict(h_u_cross=h_u_cross, h_u_target=h_u_target,
                  dom_movie=dom_movie, w_q=w_q, w_k=w_k, w_v=w_v, b_q=b_q,
                  b_k=b_k, b_v=b_v, w_o=w_o, b_o=b_o, ln1_g=ln1_g,
                  ln1_b=ln1_b, w1=w1, b1=b1, w2=w2, b2=b2, ln2_g=ln2_g,
                  ln2_b=ln2_b)
    prep, t1, t2 = host_prepare(inputs)
    nc = _get_program(B_CORE, r_blk, t1, t2)

    xc = np.asarray(h_u_cross, np.float32)
    xt = np.asarray(h_u_target, np.float32)
    xtb = np.ascontiguousarray(xt.astype(NPBF16))
    db = np.ascontiguousarray((xc - xt).astype(NPBF16))
    in_maps = []
    for c in range(N_CORES):
        m = dict(prep)
        m["xtb"] = xtb[c * B_CORE:(c + 1) * B_CORE]
        m["db"] = db[c * B_CORE:(c + 1) * B_CORE]
        in_maps.append(m)

    res = run_bass_kernel_spmd(nc, in_maps, core_ids=list(range(N_CORES)),
                               trace=trace, **run_kwargs)
    out = np.concatenate([res.results[c]["out"] for c in range(N_CORES)], axis=0)
    kernel.last_results = res
    return out.astype(np.float32)


# revision 29
# speedup vs baseline: 1.0183x; 1.0102x over previous
"""Trainium2 Bass kernel for AttentionConditionGenerator.

Reference computation (per row b of B=16384):
    kv = [h_u_cross[b], h_u_target[b]]            # (2, 1024)
    q  = dom_movie @ w_q.T + b_q                  # fixed across rows
    scores = (q/8) . k[s],  attn = softmax_2(scores)
    ctx = attn0*v0 + attn1*v1 ; y = ctx @ w_o.T + b_o
    x = LN1(dom_movie + y); h = gelu(x @ w1.T + b1)
    out = LN2(x + h @ w2.T + b2)

Algebraic folding (host, fp64, exact):
  - q row-independent -> scores fold to A @ d with d = xc - xt;
    attn0 = sigmoid(A @ d) (b_k cancels in the 2-way softmax).
  - ctx = v_t + attn0 * v_d, v_t = w_v@xt + b_v, v_d = w_v@d.
  - LN1 centering folded into the weights: with C = I - 11^T/D,
        y' = C@y = (C@w_o@w_v)@xt + (C@w_o)@(attn0*(w_v@d)) + C@bod2
    so y' arrives centered and LN1 reduces to a per-row scale:
        x = y' * rsqrt(mean(y'^2) + eps)
    x is exactly column-centered too, so LN2 sees a centered residual.

Device mapping: batch split over 8 cores (2048 rows each). Activations stay
feature-major (features on partitions) end-to-end; LN1 runs feature-major
(variance via ones-vector matmuls + a 1-row broadcast matmul), LN2 runs
row-major after cheap bf16 transposes of the final residual. All matmuls are
bf16 with fp32 PSUM accumulation.
"""

import numpy as np
import ml_dtypes

try:
    import concourse.bass as bass
except ImportError:  # pragma: no cover - path setup for fresh environments
    import sys

    for _p in ("/opt/trn_rl_repo", "/root/.axon_site/_ro/trn_rl_repo"):
        if _p not in sys.path:
            sys.path.insert(0, _p)
    import concourse.bass as bass

import concourse.mybir as mybir
import concourse.tile as tile
from concourse import bacc
from concourse.bass_utils import run_bass_kernel_spmd
from concourse.masks import make_identity

F32 = mybir.dt.float32
BF16 = mybir.dt.bfloat16
NPBF16 = ml_dtypes.bfloat16

D = 1024
H = 16
HD = 64
FFN = 4096
EPS = 1e-5
N_CORES = 8
B_TOTAL = 16384
B_CORE = B_TOTAL // N_CORES  # 2048

KT = D // 128  # 8 feature k-tiles
MT = D // 128  # 8 output m-tiles
FMT = FFN // 128  # 32 FFN m-tiles
NMG = 4  # host-side FFN1 m-group axis (1024 cols each)

AF = mybir.ActivationFunctionType


def build_program(b_core, r_blk, trivial_ln1, trivial_ln2):
    """Build and compile the per-core Bass program."""
    nb = b_core // r_blk  # row blocks
    ni = r_blk // 128  # 128-row subtiles per block
    N = r_blk  # matmul moving (free) dim

    nc = bacc.Bacc("TRN2", target_bir_lowering=False)

    # ---- DRAM I/O ------------------------------------------------------
    xtb_d = nc.dram_tensor("xtb", [b_core, D], BF16, kind="ExternalInput")
    db_d = nc.dram_tensor("db", [b_core, D], BF16, kind="ExternalInput")
    wv_d = nc.dram_tensor("wvT", [128, KT, D], BF16, kind="ExternalInput")
    wm_d = nc.dram_tensor("wmT", [128, KT, D], BF16, kind="ExternalInput")
    wo_d = nc.dram_tensor("woT", [128, KT, D], BF16, kind="ExternalInput")
    w1_d = nc.dram_tensor("w1P", [128, NMG, KT, D], BF16, kind="ExternalInput")
    w2_d = nc.dram_tensor("w2P", [128, MT, FMT, 128], BF16, kind="ExternalInput")
    at_d = nc.dram_tensor("AT", [128, KT, H], BF16, kind="ExternalInput")
    e_d = nc.dram_tensor("E", [H, MT, 128], BF16, kind="ExternalInput")
    bod_d = nc.dram_tensor("bodC", [128, MT], F32, kind="ExternalInput")
    b1_d = nc.dram_tensor("b1p", [128, FMT], F32, kind="ExternalInput")
    b2_d = nc.dram_tensor("b2p", [128, MT], F32, kind="ExternalInput")
    if not trivial_ln1:
        g1_d = nc.dram_tensor("g1p", [128, MT], F32, kind="ExternalInput")
        c1_d = nc.dram_tensor("c1p", [128, MT], F32, kind="ExternalInput")
    if not trivial_ln2:
        g2_d = nc.dram_tensor("g2", [D], F32, kind="ExternalInput")
        c2_d = nc.dram_tensor("c2", [D], F32, kind="ExternalInput")
    out_d = nc.dram_tensor("out", [b_core, D], F32, kind="ExternalOutput")

    def bcast_ap(dram):
        # [D] dram vector -> [128, D] partition-broadcast access pattern
        return bass.AP(tensor=dram, offset=0, ap=[[0, 128], [1, D]])

    with tile.TileContext(nc) as tc:
        with (
            tc.tile_pool(name="consts", bufs=1) as consts,
            tc.tile_pool(name="wres", bufs=1) as wres,
            tc.tile_pool(name="w1s", bufs=2) as w1pool,
            tc.tile_pool(name="w2s", bufs=2) as w2pool,
            tc.tile_pool(name="fm", bufs=1) as fm,
            tc.tile_pool(name="act", bufs=1) as am,
            tc.tile_pool(name="work", bufs=3) as work,
            tc.tile_pool(name="stats", bufs=2) as st,
            tc.tile_pool(name="rm", bufs=2) as rm,
            tc.tile_pool(name="ps_mm", bufs=3, space="PSUM") as ps_mm,
            tc.tile_pool(name="ps_ab", bufs=2, space="PSUM") as ps_ab,
            tc.tile_pool(name="ps_misc", bufs=2, space="PSUM") as ps_misc,
            tc.tile_pool(name="ps_tr", bufs=1, space="PSUM") as ps_tr,
        ):
            # ---- constants (at_s first: scores need it immediately;
            # the rest are loaded after the cold input transposes) -----
            at_s = consts.tile([128, KT, H], BF16)
            nc.sync.dma_start(at_s, at_d[:, :, :])
            ident = consts.tile([128, 128], BF16)
            e_s = consts.tile([H, MT, 128], BF16)
            bod_s = consts.tile([128, MT], F32)
            b1_s = consts.tile([128, FMT], F32)
            b2_s = consts.tile([128, MT], F32)
            ones_col = consts.tile([128, 1], BF16)
            nc.vector.memset(ones_col, 1.0)
            ones_row = consts.tile([1, 128], BF16)
            nc.vector.memset(ones_row, 1.0)
            eps_s = consts.tile([128, 1], F32)
            nc.vector.memset(eps_s, EPS)
            eps1 = consts.tile([1, 1], F32)
            nc.vector.memset(eps1, EPS)
            g1_s = c1_s = g2_s = c2_s = None
            if not trivial_ln1:
                g1_s = consts.tile([128, MT], F32)
                nc.sync.dma_start(g1_s, g1_d[:, :])
                c1_s = consts.tile([128, MT], F32)
                nc.sync.dma_start(c1_s, c1_d[:, :])
            if not trivial_ln2:
                g2_s = consts.tile([128, D], F32)
                nc.gpsimd.dma_start(g2_s, bcast_ap(g2_d))
                c2_s = consts.tile([128, D], F32)
                nc.gpsimd.dma_start(c2_s, bcast_ap(c2_d))

            def layernorm_rm(y_i, out_tile, g_s, c_s, last=False):
                """Row-major LayerNorm of y_i [128, D] f32 -> out_tile.
                last=True puts the normalize on DVE (the Pool impl is
                ~2x slower, which only matters on the kernel tail)."""
                stt = st.tile([128, 2, 6], F32, tag="bnst")
                nc.vector.bn_stats(stt[:, 0, :], y_i[:, 0:512])
                nc.vector.bn_stats(stt[:, 1, :], y_i[:, 512:1024])
                mv = st.tile([128, 2], F32, tag="bnmv")
                nc.vector.bn_aggr(mv, stt)
                std = st.tile([128, 1], F32, tag="bnstd")
                nc.scalar.activation(std, mv[:, 1:2], AF.Sqrt, bias=eps_s)
                rstd = st.tile([128, 1], F32, tag="bnrstd")
                nc.vector.reciprocal(rstd, std)
                eng = nc.vector if last else nc.gpsimd
                if g_s is None:
                    eng.tensor_scalar(
                        out_tile, y_i, mv[:, 0:1], rstd,
                        op0=mybir.AluOpType.subtract, op1=mybir.AluOpType.mult)
                else:
                    xn = st.tile([128, D], F32, tag="bnxn")
                    nc.gpsimd.tensor_scalar(
                        xn, y_i, mv[:, 0:1], rstd,
                        op0=mybir.AluOpType.subtract, op1=mybir.AluOpType.mult)
                    nc.gpsimd.tensor_mul(out_tile, xn, g_s)
                    nc.gpsimd.tensor_add(out_tile, out_tile, c_s)

            def load_inputs(blk):
                """DMA-transpose-load block inputs (dT first: scores need
                it). dT is split into k-halves so consumers of early
                k-tiles need not wait for the full transfer."""
                r0 = blk * r_blk
                dTa = fm.tile([128, KT // 2, N], BF16, tag="dTa", bufs=2)
                dTb = fm.tile([128, KT // 2, N], BF16, tag="dTb", bufs=2)
                xtT = fm.tile([128, KT, N], BF16, tag="xtT", bufs=2)
                for j in range(KT):
                    h = (dTa, dTb)[j // 4]
                    nc.sync.dma_start(h[:, j % 4, :],
                                      db_d[r0:r0 + N, 128 * j:128 * (j + 1)],
                                      transpose=True)
                for j in range(KT):
                    nc.sync.dma_start(xtT[:, j, :],
                                      xtb_d[r0:r0 + N, 128 * j:128 * (j + 1)],
                                      transpose=True)
                return xtT, (dTa, dTb)

            def dk(dT, k):
                return dT[k // 4][:, k % 4, :]


            def scores_mm(dT):
                """Raw attention scores A @ d -> PSUM (sigmoid deferred)."""
                psc = ps_misc.tile([H, N], F32, tag="misc")
                for k in range(KT):
                    nc.tensor.matmul(psc, at_s[:, k, :], dk(dT, k),
                                     start=(k == 0), stop=(k == KT - 1))
                return psc

            def scores_act(psc):
                attn0 = st.tile([H, N], BF16, tag="attn0", bufs=2)
                nc.scalar.activation(attn0, psc, AF.Sigmoid)
                return attn0

            def attn_u(dT, m):
                pu = ps_mm.tile([128, N], F32, tag="mm", name="pu")
                for k in range(KT):
                    nc.tensor.matmul(pu,
                                     wv_s[:, k, 128 * m:128 * (m + 1)],
                                     dk(dT, k),
                                     start=(k == 0), stop=(k == KT - 1))
                return pu

            def attn_t(t_tile, dT, attn0, m, pu=None):
                if pu is None:
                    pu = attn_u(dT, m)
                pab = ps_ab.tile([128, N], F32, name="pab")
                nc.tensor.matmul(pab, e_s[:, m, :], attn0,
                                 start=True, stop=True)
                # the BIR verifier rejects TensorTensor with two PSUM
                # operands: drain the broadcast through ACT first
                ab_s = work.tile([128, N], BF16, tag="ab", bufs=2)
                nc.scalar.copy(ab_s, pab)
                nc.vector.tensor_mul(t_tile[:, m, :], pu, ab_s)

            # ---- cold start: block 0 inputs + resident weights, ordered
            # by first use (scores->dT, u->wv, y->xtT/wm/wo) --------------
            dT0a = fm.tile([128, KT // 2, N], BF16, tag="dTa", bufs=2)
            dT0b = fm.tile([128, KT // 2, N], BF16, tag="dTb", bufs=2)
            dT0 = (dT0a, dT0b)
            xtT0 = fm.tile([128, KT, N], BF16, tag="xtT", bufs=2)
            wv_s = wres.tile([128, KT, D], BF16)
            wm_s = wres.tile([128, KT, D], BF16)
            wo_s = wres.tile([128, KT, D], BF16)
            # first input: dT halves split across the two DMA queues so
            # HWDGE descriptor dispatch (625ns each) isn't the critical
            # path into the first score/u matmuls
            for j in range(4):
                nc.sync.dma_start(dT0a[:, j, :],
                                  db_d[0:N, 128 * j:128 * (j + 1)],
                                  transpose=True)
                nc.sync.dma_start(dT0b[:, j, :],
                                  db_d[0:N, 128 * (j + 4):128 * (j + 5)],
                                  transpose=True)
            for mh in range(2):
                nc.sync.dma_start(wv_s[:, :, 512 * mh:512 * (mh + 1)],
                                  wv_d[:, :, 512 * mh:512 * (mh + 1)])
            for j in range(4):
                nc.sync.dma_start(xtT0[:, j, :],
                                  xtb_d[0:N, 128 * j:128 * (j + 1)],
                                  transpose=True)
            make_identity(nc, ident)
            nc.gpsimd.dma_start(e_s, e_d[:, :, :])
            for j in range(4, KT):
                nc.sync.dma_start(xtT0[:, j, :],
                                  xtb_d[0:N, 128 * j:128 * (j + 1)],
                                  transpose=True)
            nc.sync.dma_start(bod_s, bod_d[:, :])
            for q in range(4):
                for w_s, w_d_ in ((wm_s, wm_d), (wo_s, wo_d)):
                    nc.sync.dma_start(w_s[:, :, 256 * q:256 * (q + 1)],
                                      w_d_[:, :, 256 * q:256 * (q + 1)])
            nc.sync.dma_start(b1_s, b1_d[:, :])
            nc.sync.dma_start(b2_s, b2_d[:, :])
            nxt = (xtT0, dT0)
            nxt_at = scores_act(scores_mm(dT0))
            t_cur = am.tile([128, MT, N], BF16, tag="t", bufs=2, name="t_cur")
            pending_ln2 = None

            def emit_ln2(z_rm, r0, tail):
                """Row-major LN2 + store for one block's z_rm tiles.
                tail=True phase-batches the chains (shorter critical path
                at the very end of the kernel); otherwise per-i chains
                with the normalize on Pool (DVE stays free for the
                current block's attention)."""
                g_s = None if trivial_ln2 else g2_s
                c_s = None if trivial_ln2 else c2_s
                if not tail:
                    for i in range(ni):
                        layernorm_rm(z_rm[i], z_rm[i], g_s, c_s)
                        nc.gpsimd.dma_start(
                            out_d[r0 + 128 * i:r0 + 128 * (i + 1), :], z_rm[i])
                    return
                stts, mvs, stds, rstds = [], [], [], []
                for i in range(ni):
                    stt = st.tile([128, 2, 6], F32, tag="bnstT", bufs=ni,
                                  name="stt")
                    nc.vector.bn_stats(stt[:, 0, :], z_rm[i][:, 0:512])
                    nc.vector.bn_stats(stt[:, 1, :], z_rm[i][:, 512:1024])
                    stts.append(stt)
                for i in range(ni):
                    mv = st.tile([128, 2], F32, tag="bnmvT", bufs=ni, name="mv")
                    nc.vector.bn_aggr(mv, stts[i])
                    mvs.append(mv)
                for i in range(ni):
                    std = st.tile([128, 1], F32, tag="bnstdT", bufs=ni,
                                  name="std_i")
                    nc.scalar.activation(std, mvs[i][:, 1:2], AF.Sqrt,
                                         bias=eps_s)
                    stds.append(std)
                for i in range(ni):
                    rstd = st.tile([128, 1], F32, tag="bnrstdT", bufs=ni,
                                   name="rstd_i")
                    nc.vector.reciprocal(rstd, stds[i])
                    rstds.append(rstd)
                for i in range(ni):
                    nc.vector.tensor_scalar(
                        z_rm[i], z_rm[i], mvs[i][:, 0:1], rstds[i],
                        op0=mybir.AluOpType.subtract,
                        op1=mybir.AluOpType.mult)
                    if g_s is not None:
                        nc.vector.tensor_mul(z_rm[i], z_rm[i], g_s)
                        nc.vector.tensor_add(z_rm[i], z_rm[i], c_s)
                    # alternate queues so the final stores overlap; SP is
                    # idle here (no more weight traffic)
                    q = nc.sync if i % 2 == 0 else nc.gpsimd
                    q.dma_start(
                        out_d[r0 + 128 * i:r0 + 128 * (i + 1), :], z_rm[i])

            for blk in range(nb):
                r0 = blk * r_blk
                xtT, dT = nxt
                attn0 = nxt_at

                # next block's inputs: the sync DMA queue is empty here
                if blk + 1 < nb:
                    nxt = load_inputs(blk + 1)

                # ---- attention: t = attn0 * (w_v @ d) -------------------
                # (m=0,1 of this block were pulled into the previous
                # block's LN1 window as PE filler)
                for m in range(0 if blk == 0 else 2, MT):
                    attn_t(t_cur, dT, attn0, m)

                # previous block's LN2 + store, emitted here so its DVE
                # chains fill this block's y-group window instead of
                # racing this block's t-mults
                if pending_ln2 is not None:
                    emit_ln2(*pending_ln2, tail=False)
                    pending_ln2 = None

                # ---- y' = (C w_o w_v)@xt + (C w_o)@t + bodC  (centered) -
                yp = am.tile([128, MT, N], BF16, tag="yp")
                ss = ps_misc.tile([1, N], F32, tag="misc")
                for m in range(MT):
                    py = ps_mm.tile([128, N], F32, tag="mm")
                    for k in range(KT):
                        nc.tensor.matmul(py,
                                         wm_s[:, k, 128 * m:128 * (m + 1)],
                                         xtT[:, k, :],
                                         start=(k == 0), stop=False)
                    for k in range(KT):
                        nc.tensor.matmul(py,
                                         wo_s[:, k, 128 * m:128 * (m + 1)],
                                         t_cur[:, k, :],
                                         start=False, stop=(k == KT - 1))
                    nc.scalar.activation(yp[:, m, :], py, AF.Identity,
                                         bias=bod_s[:, m:m + 1])
                    y2 = work.tile([128, N], BF16, tag="y2", bufs=2)
                    nc.scalar.activation(y2, py, AF.Square,
                                         bias=bod_s[:, m:m + 1])
                    nc.tensor.matmul(ss, ones_col, y2,
                                     start=(m == 0), stop=(m == MT - 1))

                # prefetch the first two FFN1 weight slices now
                w1_pre = []
                for mg in range(2):
                    w1_s = w1pool.tile([128, KT, 512], BF16, tag="wA",
                                       name="w1_s")
                    nc.sync.dma_start(
                        w1_s,
                        w1_d[:, mg // 2, :, 512 * (mg % 2):512 * (mg % 2 + 1)])
                    w1_pre.append(w1_s)

                # ---- LN1 scale: x = y' * rsqrt(mean(y'^2)+eps) ----------
                # The sqrt->recip->bcast chain has no PE work of its own;
                # next block's scores + attention m=0,1 fill the bubble.
                std = st.tile([1, N], F32, tag="std")
                nc.scalar.activation(std, ss, AF.Sqrt, bias=eps1, scale=1.0 / D)
                rstd_bf = st.tile([1, N], BF16, tag="rstdb")
                with nc.allow_low_precision(
                        reason="bf16 rstd: 0.1%% scale noise, LN2 renormalizes"):
                    nc.vector.reciprocal(rstd_bf, std)
                if blk + 1 < nb:
                    nxt_at = scores_act(scores_mm(nxt[1]))
                    t_nxt = am.tile([128, MT, N], BF16, tag="t", bufs=2,
                                    name="t_nxt")
                    pu0 = attn_u(nxt[1], 0)
                    pu1 = attn_u(nxt[1], 1)
                    attn_t(t_nxt, nxt[1], nxt_at, 0, pu=pu0)
                    attn_t(t_nxt, nxt[1], nxt_at, 1, pu=pu1)
                pr1 = ps_misc.tile([128, N], F32, tag="misc")
                nc.tensor.matmul(pr1, ones_row, rstd_bf, start=True, stop=True)
                r1_s = work.tile([128, N], BF16, tag="r1", bufs=2)
                nc.scalar.copy(r1_s, pr1)
                x_s = am.tile([128, MT, N], BF16, tag="x")

                def x_mults():
                    for m in range(MT):
                        if trivial_ln1:
                            # all-bf16 SBUF operands: 2x DVE mode
                            nc.vector.tensor_mul(x_s[:, m, :], yp[:, m, :],
                                                 r1_s)
                        else:
                            xm = work.tile([128, N], F32, tag="xm")
                            nc.vector.tensor_mul(xm, yp[:, m, :], r1_s)
                            nc.vector.tensor_scalar(
                                x_s[:, m, :], xm, g1_s[:, m:m + 1],
                                c1_s[:, m:m + 1],
                                op0=mybir.AluOpType.mult,
                                op1=mybir.AluOpType.add)

                if blk + 1 < nb or not trivial_ln1:
                    x_mults()

                # ---- FFN1: h = gelu(w1 @ x + b1) ------------------------
                # Last block: no next-block filler exists for the LN1
                # chain, so run the matmuls on raw y' and fold the
                # per-column rstd scale in after the matmul (exact:
                # w1 @ (y'*r) = (w1 @ y') * r). PE never waits on rstd.
                last_blk = blk == nb - 1 and trivial_ln1
                hT = am.tile([128, FMT, N], BF16, tag="hT")
                for mg in range(8):
                    if mg < 2:
                        w1_s = w1_pre[mg]
                    else:
                        w1_s = w1pool.tile([128, KT, 512], BF16, tag="wA",
                                           name="w1_s")
                        nc.sync.dma_start(
                            w1_s,
                            w1_d[:, mg // 2, :,
                                 512 * (mg % 2):512 * (mg % 2 + 1)])
                    for mm in range(4):
                        m = mg * 4 + mm
                        pm = ps_mm.tile([128, N], F32, tag="mm")
                        f1_src = yp if last_blk else x_s
                        for k in range(KT):
                            nc.tensor.matmul(pm,
                                             w1_s[:, k, 128 * mm:128 * (mm + 1)],
                                             f1_src[:, k, :],
                                             start=(k == 0), stop=(k == KT - 1))
                        if last_blk:
                            tg = work.tile([128, N], BF16, tag="fz", bufs=2,
                                           name="tg")
                            nc.vector.tensor_mul(tg, pm, r1_s)
                            nc.scalar.activation(hT[:, m, :], tg, AF.Gelu,
                                                 bias=b1_s[:, m:m + 1])
                        else:
                            nc.scalar.activation(hT[:, m, :], pm, AF.Gelu,
                                                 bias=b1_s[:, m:m + 1])
                if last_blk:
                    # residual x needed only from FFN2 onward
                    x_mults()

                # ---- FFN2 + residual + bf16 transpose -------------------
                # w2 stream runs one half-slice ahead of the consuming
                # matmuls (bufs=2: one in use, one loading).
                def load_w2(m, kh):
                    w2_s = w2pool.tile([128, 16, 128], BF16, tag="w2s",
                                       name="w2_s")
                    nc.sync.dma_start(w2_s,
                                      w2_d[:, m, 16 * kh:16 * (kh + 1), :])
                    return w2_s

                z_rm = [rm.tile([128, D], F32, tag="z_rm", bufs=ni,
                                name="z_rm")
                        for _ in range(ni)]
                w2_nxt = load_w2(0, 0)
                for m in range(MT):
                    pm = ps_mm.tile([128, N], F32, tag="mm")
                    for kh in range(2):
                        w2_s = w2_nxt
                        if not (m == MT - 1 and kh == 1):
                            w2_nxt = load_w2(m + kh, (kh + 1) % 2)
                        for kk in range(16):
                            k = 16 * kh + kk
                            nc.tensor.matmul(pm, w2_s[:, kk, :], hT[:, k, :],
                                             start=(k == 0),
                                             stop=(k == FMT - 1))
                    fz = work.tile([128, N], BF16, tag="fz", bufs=2)
                    nc.scalar.activation(fz, pm, AF.Identity,
                                         bias=b2_s[:, m:m + 1])
                    zt = work.tile([128, N], BF16, tag="zt")
                    nc.vector.tensor_add(zt, x_s[:, m, :], fz)
                    ptr = ps_tr.tile([128, ni * 128], BF16)
                    for i in range(ni):
                        nc.tensor.transpose(ptr[:, 128 * i:128 * (i + 1)],
                                            zt[:, 128 * i:128 * (i + 1)], ident)
                    for i in range(ni):
                        nc.scalar.copy(z_rm[i][:, 128 * m:128 * (m + 1)],
                                       ptr[:, 128 * i:128 * (i + 1)])

                pending_ln2 = (z_rm, r0)
                if blk + 1 < nb:
                    t_cur = t_nxt
            emit_ln2(*pending_ln2, tail=True)

    nc.compile()
    return nc


def host_prepare(inputs):
    """Fold parameters and lay out weights for the device (all O(params))."""
    f64 = {k: np.asarray(inputs[k], dtype=np.float64)
           for k in ("dom_movie", "w_q", "w_k", "w_v", "b_q", "w_o", "b_o",
                     "b_v")}
    qs = (f64["dom_movie"] @ f64["w_q"].T + f64["b_q"]) / np.sqrt(HD)  # (1, D)
    qh = qs.reshape(H, HD)
    A = np.einsum("hd,hdD->hD", qh, f64["w_k"].reshape(H, HD, D))  # (H, D)
    bod2 = f64["b_o"] + f64["dom_movie"][0] + f64["w_o"] @ f64["b_v"]  # (D,)

    # LN centering folded into the output projection: C = I - 11^T/D
    woC = f64["w_o"] - f64["w_o"].mean(axis=0, keepdims=True)   # C @ w_o
    M = woC @ f64["w_v"]                                        # C w_o w_v
    bodC = bod2 - bod2.mean()                                   # C @ bod2

    E = np.zeros((H, MT, 128), np.float32)
    for m in range(MT):
        for p in range(128):
            E[2 * m + p // 64, m, p] = 1.0

    w1 = np.asarray(inputs["w1"], np.float32)
    w2 = np.asarray(inputs["w2"], np.float32)

    def fm_weight(wT):  # wT (d_in, d_out) -> [128, d_in/128, d_out]
        return np.ascontiguousarray(
            wT.reshape(-1, 128, wT.shape[1]).transpose(1, 0, 2)).astype(NPBF16)

    prep = {
        "wvT": fm_weight(np.asarray(f64["w_v"], np.float32).T),
        "wmT": fm_weight(np.asarray(M, np.float32).T),
        "woT": fm_weight(np.asarray(woC, np.float32).T),
        "w1P": np.ascontiguousarray(
            w1.T.reshape(KT, 128, NMG, D).transpose(1, 2, 0, 3)).astype(NPBF16),
        "w2P": np.ascontiguousarray(
            w2.T.reshape(FMT, 128, MT, 128).transpose(1, 2, 0, 3)).astype(NPBF16),
        "AT": np.ascontiguousarray(
            A.T.reshape(KT, 128, H).transpose(1, 0, 2)).astype(NPBF16),
        "E": E.astype(NPBF16),
        "bodC": np.ascontiguousarray(
            bodC.reshape(MT, 128).T).astype(np.float32),
        "b1p": np.ascontiguousarray(
            np.asarray(inputs["b1"], np.float64).reshape(FMT, 128).T
        ).astype(np.float32),
        "b2p": np.ascontiguousarray(
            np.asarray(inputs["b2"], np.float64).reshape(MT, 128).T
        ).astype(np.float32),
    }
    trivial_ln1 = bool(np.all(np.asarray(inputs["ln1_g"]) == 1.0)
                       and np.all(np.asarray(inputs["ln1_b"]) == 0.0))
    trivial_ln2 = bool(np.all(np.asarray(inputs["ln2_g"]) == 1.0)
                       and np.all(np.asarray(inputs["ln2_b"]) == 0.0))
    if not trivial_ln1:
        prep["g1p"] = np.ascontiguousarray(
            np.asarray(inputs["ln1_g"], np.float64).reshape(MT, 128).T
        ).astype(np.float32)
        prep["c1p"] = np.ascontiguousarray(
            np.asarray(inputs["ln1_b"], np.float64).reshape(MT, 128).T
        ).astype(np.float32)
    if not trivial_ln2:
        prep["g2"] = np.asarray(inputs["ln2_g"], np.float32)
        prep["c2"] = np.asarray(inputs["ln2_b"], np.float32)
    return prep, trivial_ln1, trivial_ln2


_PROGRAM_CACHE = {}


def _get_program(b_core, r_blk, t1, t2):
    key = (b_core, r_blk, t1, t2)
    if key not in _PROGRAM_CACHE:
        _PROGRAM_CACHE[key] = build_program(b_core, r_blk, t1, t2)
    return _PROGRAM_CACHE[key]


def kernel(h_u_cross, h_u_target, dom_movie, w_q, w_k, w_v, b_q, b_k, b_v,
           w_o, b_o, ln1_g, ln1_b, w1, b1, w2, b2, ln2_g, ln2_b,
           trace=False, r_blk=512, **run_kwargs):
    inputs = dict(h_u_cross=h_u_cross, h_u_target=h_u_target,
                  dom_movie=dom_movie, w_q=w_q, w_k=w_k, w_v=w_v, b_q=b_q,
                  b_k=b_k, b_v=b_v, w_o=w_o, b_o=b_o, ln1_g=ln1_g,
                  ln1_b=ln1_b, w1=w1, b1=b1, w2=w2, b2=b2, ln2_g=ln2_g,
                  ln2_b=ln2_b)
    prep, t1, t2 = host_prepare(inputs)
    nc = _get_program(B_CORE, r_blk, t1, t2)

    xc = np.asarray(h_u_cross, np.float32)
    xt = np.asarray(h_u_target, np.float32)
    xtb = np.ascontiguousarray(xt.astype(NPBF16))
    db = np.ascontiguousarray((xc - xt).astype(NPBF16))
    in_maps = []
    for c in range(N_CORES):
        m = dict(prep)
        m["xtb"] = xtb[c * B_CORE:(c + 1) * B_CORE]
        m["db"] = db[c * B_CORE:(c + 1) * B_CORE]
        in_maps.append(m)

    res = run_bass_kernel_spmd(nc, in_maps, core_ids=list(range(N_CORES)),
                               trace=trace, **run_kwargs)
    out = np.concatenate([res.results[c]["out"] for c in range(N_CORES)], axis=0)
    kernel.last_results = res
    return out.astype(np.float32)


# revision 31
# speedup vs baseline: 1.0194x; 1.0010x over previous
"""Trainium2 Bass kernel for AttentionConditionGenerator.

Reference computation (per row b of B=16384):
    kv = [h_u_cross[b], h_u_target[b]]            # (2, 1024)
    q  = dom_movie @ w_q.T + b_q                  # fixed across rows
    scores = (q/8) . k[s],  attn = softmax_2(scores)
    ctx = attn0*v0 + attn1*v1 ; y = ctx @ w_o.T + b_o
    x = LN1(dom_movie + y); h = gelu(x @ w1.T + b1)
    out = LN2(x + h @ w2.T + b2)

Algebraic folding (host, fp64, exact):
  - q row-independent -> scores fold to A @ d with d = xc - xt;
    attn0 = sigmoid(A @ d) (b_k cancels in the 2-way softmax).
  - ctx = v_t + attn0 * v_d, v_t = w_v@xt + b_v, v_d = w_v@d.
  - LN1 centering folded into the weights: with C = I - 11^T/D,
        y' = C@y = (C@w_o@w_v)@xt + (C@w_o)@(attn0*(w_v@d)) + C@bod2
    so y' arrives centered and LN1 reduces to a per-row scale:
        x = y' * rsqrt(mean(y'^2) + eps)
    x is exactly column-centered too, so LN2 sees a centered residual.

Device mapping: batch split over 8 cores (2048 rows each). Activations stay
feature-major (features on partitions) end-to-end; LN1 runs feature-major
(variance via ones-vector matmuls + a 1-row broadcast matmul), LN2 runs
row-major after cheap bf16 transposes of the final residual. All matmuls are
bf16 with fp32 PSUM accumulation.
"""

import numpy as np
import ml_dtypes

try:
    import concourse.bass as bass
except ImportError:  # pragma: no cover - path setup for fresh environments
    import sys

    for _p in ("/opt/trn_rl_repo", "/root/.axon_site/_ro/trn_rl_repo"):
        if _p not in sys.path:
            sys.path.insert(0, _p)
    import concourse.bass as bass

import concourse.mybir as mybir
import concourse.tile as tile
from concourse import bacc
from concourse.bass_utils import run_bass_kernel_spmd
from concourse.masks import make_identity

F32 = mybir.dt.float32
BF16 = mybir.dt.bfloat16
NPBF16 = ml_dtypes.bfloat16

D = 1024
H = 16
HD = 64
FFN = 4096
EPS = 1e-5
N_CORES = 8
B_TOTAL = 16384
B_CORE = B_TOTAL // N_CORES  # 2048

KT = D // 128  # 8 feature k-tiles
MT = D // 128  # 8 output m-tiles
FMT = FFN // 128  # 32 FFN m-tiles
NMG = 4  # host-side FFN1 m-group axis (1024 cols each)

AF = mybir.ActivationFunctionType


def build_program(b_core, r_blk, trivial_ln1, trivial_ln2):
    """Build and compile the per-core Bass program."""
    nb = b_core // r_blk  # row blocks
    ni = r_blk // 128  # 128-row subtiles per block
    N = r_blk  # matmul moving (free) dim

    nc = bacc.Bacc("TRN2", target_bir_lowering=False)

    # ---- DRAM I/O ------------------------------------------------------
    xtb_d = nc.dram_tensor("xtb", [b_core, D], BF16, kind="ExternalInput")
    db_d = nc.dram_tensor("db", [b_core, D], BF16, kind="ExternalInput")
    wv_d = nc.dram_tensor("wvT", [128, KT, D], BF16, kind="ExternalInput")
    wm_d = nc.dram_tensor("wmT", [128, KT, D], BF16, kind="ExternalInput")
    wo_d = nc.dram_tensor("woT", [128, KT, D], BF16, kind="ExternalInput")
    w1_d = nc.dram_tensor("w1P", [128, NMG, KT, D], BF16, kind="ExternalInput")
    w2_d = nc.dram_tensor("w2P", [128, MT, FMT, 128], BF16, kind="ExternalInput")
    at_d = nc.dram_tensor("AT", [128, KT, H], BF16, kind="ExternalInput")
    e_d = nc.dram_tensor("E", [H, MT, 128], BF16, kind="ExternalInput")
    bod_d = nc.dram_tensor("bodC", [128, MT], F32, kind="ExternalInput")
    b1_d = nc.dram_tensor("b1p", [128, FMT], F32, kind="ExternalInput")
    b2_d = nc.dram_tensor("b2p", [128, MT], F32, kind="ExternalInput")
    if not trivial_ln1:
        g1_d = nc.dram_tensor("g1p", [128, MT], F32, kind="ExternalInput")
        c1_d = nc.dram_tensor("c1p", [128, MT], F32, kind="ExternalInput")
    if not trivial_ln2:
        g2_d = nc.dram_tensor("g2", [D], F32, kind="ExternalInput")
        c2_d = nc.dram_tensor("c2", [D], F32, kind="ExternalInput")
    out_d = nc.dram_tensor("out", [b_core, D], F32, kind="ExternalOutput")

    def bcast_ap(dram):
        # [D] dram vector -> [128, D] partition-broadcast access pattern
        return bass.AP(tensor=dram, offset=0, ap=[[0, 128], [1, D]])

    with tile.TileContext(nc) as tc:
        with (
            tc.tile_pool(name="consts", bufs=1) as consts,
            tc.tile_pool(name="wres", bufs=1) as wres,
            tc.tile_pool(name="w1s", bufs=2) as w1pool,
            tc.tile_pool(name="w2s", bufs=2) as w2pool,
            tc.tile_pool(name="fm", bufs=1) as fm,
            tc.tile_pool(name="act", bufs=1) as am,
            tc.tile_pool(name="work", bufs=3) as work,
            tc.tile_pool(name="stats", bufs=2) as st,
            tc.tile_pool(name="rm", bufs=2) as rm,
            tc.tile_pool(name="ps_mm", bufs=3, space="PSUM") as ps_mm,
            tc.tile_pool(name="ps_ab", bufs=2, space="PSUM") as ps_ab,
            tc.tile_pool(name="ps_misc", bufs=2, space="PSUM") as ps_misc,
            tc.tile_pool(name="ps_tr", bufs=1, space="PSUM") as ps_tr,
        ):
            # ---- constants (at_s first: scores need it immediately;
            # the rest are loaded after the cold input transposes) -----
            at_s = consts.tile([128, KT, H], BF16)
            nc.sync.dma_start(at_s, at_d[:, :, :])
            ident = consts.tile([128, 128], BF16)
            e_s = consts.tile([H, MT, 128], BF16)
            bod_s = consts.tile([128, MT], F32)
            b1_s = consts.tile([128, FMT], F32)
            b2_s = consts.tile([128, MT], F32)
            ones_col = consts.tile([128, 1], BF16)
            nc.vector.memset(ones_col, 1.0)
            ones_row = consts.tile([1, 128], BF16)
            nc.vector.memset(ones_row, 1.0)
            eps_s = consts.tile([128, 1], F32)
            nc.vector.memset(eps_s, EPS)
            eps1 = consts.tile([1, 1], F32)
            nc.vector.memset(eps1, EPS)
            g1_s = c1_s = g2_s = c2_s = None
            if not trivial_ln1:
                g1_s = consts.tile([128, MT], F32)
                nc.sync.dma_start(g1_s, g1_d[:, :])
                c1_s = consts.tile([128, MT], F32)
                nc.sync.dma_start(c1_s, c1_d[:, :])
            if not trivial_ln2:
                g2_s = consts.tile([128, D], F32)
                nc.gpsimd.dma_start(g2_s, bcast_ap(g2_d))
                c2_s = consts.tile([128, D], F32)
                nc.gpsimd.dma_start(c2_s, bcast_ap(c2_d))

            def layernorm_rm(y_i, out_tile, g_s, c_s, last=False):
                """Row-major LayerNorm of y_i [128, D] f32 -> out_tile.
                last=True puts the normalize on DVE (the Pool impl is
                ~2x slower, which only matters on the kernel tail)."""
                stt = st.tile([128, 2, 6], F32, tag="bnst")
                nc.vector.bn_stats(stt[:, 0, :], y_i[:, 0:512])
                nc.vector.bn_stats(stt[:, 1, :], y_i[:, 512:1024])
                mv = st.tile([128, 2], F32, tag="bnmv")
                nc.vector.bn_aggr(mv, stt)
                std = st.tile([128, 1], F32, tag="bnstd")
                nc.scalar.activation(std, mv[:, 1:2], AF.Sqrt, bias=eps_s)
                rstd = st.tile([128, 1], F32, tag="bnrstd")
                nc.vector.reciprocal(rstd, std)
                eng = nc.vector if last else nc.gpsimd
                if g_s is None:
                    eng.tensor_scalar(
                        out_tile, y_i, mv[:, 0:1], rstd,
                        op0=mybir.AluOpType.subtract, op1=mybir.AluOpType.mult)
                else:
                    xn = st.tile([128, D], F32, tag="bnxn")
                    nc.gpsimd.tensor_scalar(
                        xn, y_i, mv[:, 0:1], rstd,
                        op0=mybir.AluOpType.subtract, op1=mybir.AluOpType.mult)
                    nc.gpsimd.tensor_mul(out_tile, xn, g_s)
                    nc.gpsimd.tensor_add(out_tile, out_tile, c_s)

            def load_inputs(blk):
                """DMA-transpose-load block inputs (dT first: scores need
                it). dT is split into k-halves so consumers of early
                k-tiles need not wait for the full transfer."""
                r0 = blk * r_blk
                dTa = fm.tile([128, KT // 2, N], BF16, tag="dTa", bufs=2)
                dTb = fm.tile([128, KT // 2, N], BF16, tag="dTb", bufs=2)
                xtT = fm.tile([128, KT, N], BF16, tag="xtT", bufs=2)
                nc.sync.dma_start(dTa, db_d[r0:r0 + N, 0:512], transpose=True)
                nc.sync.dma_start(dTb, db_d[r0:r0 + N, 512:1024],
                                  transpose=True)
                nc.sync.dma_start(xtT[:, 0:4, :], xtb_d[r0:r0 + N, 0:512],
                                  transpose=True)
                nc.sync.dma_start(xtT[:, 4:8, :], xtb_d[r0:r0 + N, 512:1024],
                                  transpose=True)
                return xtT, (dTa, dTb)

            def dk(dT, k):
                return dT[k // 4][:, k % 4, :]


            def scores_mm(dT):
                """Raw attention scores A @ d -> PSUM (sigmoid deferred)."""
                psc = ps_misc.tile([H, N], F32, tag="misc")
                for k in range(KT):
                    nc.tensor.matmul(psc, at_s[:, k, :], dk(dT, k),
                                     start=(k == 0), stop=(k == KT - 1))
                return psc

            def scores_act(psc):
                attn0 = st.tile([H, N], BF16, tag="attn0", bufs=2)
                nc.scalar.activation(attn0, psc, AF.Sigmoid)
                return attn0

            def attn_u(dT, m):
                pu = ps_mm.tile([128, N], F32, tag="mm", name="pu")
                for k in range(KT):
                    nc.tensor.matmul(pu,
                                     wv_s[:, k, 128 * m:128 * (m + 1)],
                                     dk(dT, k),
                                     start=(k == 0), stop=(k == KT - 1))
                return pu

            def attn_t(t_tile, dT, attn0, m, pu=None):
                if pu is None:
                    pu = attn_u(dT, m)
                pab = ps_ab.tile([128, N], F32, name="pab")
                nc.tensor.matmul(pab, e_s[:, m, :], attn0,
                                 start=True, stop=True)
                # the BIR verifier rejects TensorTensor with two PSUM
                # operands: drain the broadcast through ACT first
                ab_s = work.tile([128, N], BF16, tag="ab", bufs=2)
                nc.scalar.copy(ab_s, pab)
                nc.vector.tensor_mul(t_tile[:, m, :], pu, ab_s)

            # ---- cold start: block 0 inputs + resident weights, ordered
            # by first use (scores->dT, u->wv, y->xtT/wm/wo) --------------
            dT0a = fm.tile([128, KT // 2, N], BF16, tag="dTa", bufs=2)
            dT0b = fm.tile([128, KT // 2, N], BF16, tag="dTb", bufs=2)
            dT0 = (dT0a, dT0b)
            xtT0 = fm.tile([128, KT, N], BF16, tag="xtT", bufs=2)
            wv_s = wres.tile([128, KT, D], BF16)
            wm_s = wres.tile([128, KT, D], BF16)
            wo_s = wres.tile([128, KT, D], BF16)
            # first input: dT halves split across the two DMA queues so
            # HWDGE descriptor dispatch (625ns each) isn't the critical
            # path into the first score/u matmuls
            nc.sync.dma_start(dT0a, db_d[0:N, 0:512], transpose=True)
            nc.sync.dma_start(dT0b, db_d[0:N, 512:1024], transpose=True)
            for mh in range(2):
                nc.sync.dma_start(wv_s[:, :, 512 * mh:512 * (mh + 1)],
                                  wv_d[:, :, 512 * mh:512 * (mh + 1)])
            nc.sync.dma_start(xtT0[:, 0:4, :], xtb_d[0:N, 0:512],
                              transpose=True)
            make_identity(nc, ident)
            nc.gpsimd.dma_start(e_s, e_d[:, :, :])
            nc.sync.dma_start(xtT0[:, 4:8, :], xtb_d[0:N, 512:1024],
                              transpose=True)
            nc.sync.dma_start(bod_s, bod_d[:, :])
            for q in range(4):
                for w_s, w_d_ in ((wm_s, wm_d), (wo_s, wo_d)):
                    nc.sync.dma_start(w_s[:, :, 256 * q:256 * (q + 1)],
                                      w_d_[:, :, 256 * q:256 * (q + 1)])
            nc.sync.dma_start(b1_s, b1_d[:, :])
            nc.sync.dma_start(b2_s, b2_d[:, :])
            nxt = (xtT0, dT0)
            nxt_at = scores_act(scores_mm(dT0))
            t_cur = am.tile([128, MT, N], BF16, tag="t", bufs=2, name="t_cur")
            pending_ln2 = None

            def emit_ln2(z_rm, r0, tail):
                """Row-major LN2 + store for one block's z_rm tiles.
                tail=True phase-batches the chains (shorter critical path
                at the very end of the kernel); otherwise per-i chains
                with the normalize on Pool (DVE stays free for the
                current block's attention)."""
                g_s = None if trivial_ln2 else g2_s
                c_s = None if trivial_ln2 else c2_s
                if not tail:
                    for i in range(ni):
                        layernorm_rm(z_rm[i], z_rm[i], g_s, c_s)
                        nc.gpsimd.dma_start(
                            out_d[r0 + 128 * i:r0 + 128 * (i + 1), :], z_rm[i])
                    return
                stts, mvs, stds, rstds = [], [], [], []
                for i in range(ni):
                    stt = st.tile([128, 2, 6], F32, tag="bnstT", bufs=ni,
                                  name="stt")
                    nc.vector.bn_stats(stt[:, 0, :], z_rm[i][:, 0:512])
                    nc.vector.bn_stats(stt[:, 1, :], z_rm[i][:, 512:1024])
                    stts.append(stt)
                for i in range(ni):
                    mv = st.tile([128, 2], F32, tag="bnmvT", bufs=ni, name="mv")
                    nc.vector.bn_aggr(mv, stts[i])
                    mvs.append(mv)
                for i in range(ni):
                    std = st.tile([128, 1], F32, tag="bnstdT", bufs=ni,
                                  name="std_i")
                    nc.scalar.activation(std, mvs[i][:, 1:2], AF.Sqrt,
                                         bias=eps_s)
                    stds.append(std)
                for i in range(ni):
                    rstd = st.tile([128, 1], F32, tag="bnrstdT", bufs=ni,
                                   name="rstd_i")
                    nc.vector.reciprocal(rstd, stds[i])
                    rstds.append(rstd)
                for i in range(ni):
                    nc.vector.tensor_scalar(
                        z_rm[i], z_rm[i], mvs[i][:, 0:1], rstds[i],
                        op0=mybir.AluOpType.subtract,
                        op1=mybir.AluOpType.mult)
                    if g_s is not None:
                        nc.vector.tensor_mul(z_rm[i], z_rm[i], g_s)
                        nc.vector.tensor_add(z_rm[i], z_rm[i], c_s)
                    # alternate queues so the final stores overlap; SP is
                    # idle here (no more weight traffic)
                    q = nc.sync if i % 2 == 0 else nc.gpsimd
                    q.dma_start(
                        out_d[r0 + 128 * i:r0 + 128 * (i + 1), :], z_rm[i])

            for blk in range(nb):
                r0 = blk * r_blk
                xtT, dT = nxt
                attn0 = nxt_at

                # next block's inputs: the sync DMA queue is empty here
                if blk + 1 < nb:
                    nxt = load_inputs(blk + 1)

                # ---- attention: t = attn0 * (w_v @ d) -------------------
                # (m=0,1 of this block were pulled into the previous
                # block's LN1 window as PE filler)
                for m in range(0 if blk == 0 else 2, MT):
                    attn_t(t_cur, dT, attn0, m)

                # previous block's LN2 + store, emitted here so its DVE
                # chains fill this block's y-group window instead of
                # racing this block's t-mults
                if pending_ln2 is not None:
                    emit_ln2(*pending_ln2, tail=False)
                    pending_ln2 = None

                # ---- y' = (C w_o w_v)@xt + (C w_o)@t + bodC  (centered) -
                yp = am.tile([128, MT, N], BF16, tag="yp")
                ss = ps_misc.tile([1, N], F32, tag="misc")
                for m in range(MT):
                    py = ps_mm.tile([128, N], F32, tag="mm")
                    for k in range(KT):
                        nc.tensor.matmul(py,
                                         wm_s[:, k, 128 * m:128 * (m + 1)],
                                         xtT[:, k, :],
                                         start=(k == 0), stop=False)
                    for k in range(KT):
                        nc.tensor.matmul(py,
                                         wo_s[:, k, 128 * m:128 * (m + 1)],
                                         t_cur[:, k, :],
                                         start=False, stop=(k == KT - 1))
                    nc.scalar.activation(yp[:, m, :], py, AF.Identity,
                                         bias=bod_s[:, m:m + 1])
                    y2 = work.tile([128, N], BF16, tag="y2", bufs=2)
                    nc.scalar.activation(y2, py, AF.Square,
                                         bias=bod_s[:, m:m + 1])
                    nc.tensor.matmul(ss, ones_col, y2,
                                     start=(m == 0), stop=(m == MT - 1))

                # prefetch the first two FFN1 weight slices now
                w1_pre = []
                for mg in range(2):
                    w1_s = w1pool.tile([128, KT, 512], BF16, tag="wA",
                                       name="w1_s")
                    nc.sync.dma_start(
                        w1_s,
                        w1_d[:, mg // 2, :, 512 * (mg % 2):512 * (mg % 2 + 1)])
                    w1_pre.append(w1_s)

                # ---- LN1 scale: x = y' * rsqrt(mean(y'^2)+eps) ----------
                # The sqrt->recip->bcast chain has no PE work of its own;
                # next block's scores + attention m=0,1 fill the bubble.
                std = st.tile([1, N], F32, tag="std")
                nc.scalar.activation(std, ss, AF.Sqrt, bias=eps1, scale=1.0 / D)
                rstd_bf = st.tile([1, N], BF16, tag="rstdb")
                with nc.allow_low_precision(
                        reason="bf16 rstd: 0.1%% scale noise, LN2 renormalizes"):
                    nc.vector.reciprocal(rstd_bf, std)
                if blk + 1 < nb:
                    nxt_at = scores_act(scores_mm(nxt[1]))
                    t_nxt = am.tile([128, MT, N], BF16, tag="t", bufs=2,
                                    name="t_nxt")
                    pu0 = attn_u(nxt[1], 0)
                    pu1 = attn_u(nxt[1], 1)
                    attn_t(t_nxt, nxt[1], nxt_at, 0, pu=pu0)
                    attn_t(t_nxt, nxt[1], nxt_at, 1, pu=pu1)
                pr1 = ps_misc.tile([128, N], F32, tag="misc")
                nc.tensor.matmul(pr1, ones_row, rstd_bf, start=True, stop=True)
                r1_s = work.tile([128, N], BF16, tag="r1", bufs=2)
                nc.scalar.copy(r1_s, pr1)
                x_s = am.tile([128, MT, N], BF16, tag="x")

                def x_mults():
                    for m in range(MT):
                        if trivial_ln1:
                            # all-bf16 SBUF operands: 2x DVE mode
                            nc.vector.tensor_mul(x_s[:, m, :], yp[:, m, :],
                                                 r1_s)
                        else:
                            xm = work.tile([128, N], F32, tag="xm")
                            nc.vector.tensor_mul(xm, yp[:, m, :], r1_s)
                            nc.vector.tensor_scalar(
                                x_s[:, m, :], xm, g1_s[:, m:m + 1],
                                c1_s[:, m:m + 1],
                                op0=mybir.AluOpType.mult,
                                op1=mybir.AluOpType.add)

                if blk + 1 < nb or not trivial_ln1:
                    x_mults()

                # ---- FFN1: h = gelu(w1 @ x + b1) ------------------------
                # Last block: no next-block filler exists for the LN1
                # chain, so run the matmuls on raw y' and fold the
                # per-column rstd scale in after the matmul (exact:
                # w1 @ (y'*r) = (w1 @ y') * r). PE never waits on rstd.
                last_blk = blk == nb - 1 and trivial_ln1
                hT = am.tile([128, FMT, N], BF16, tag="hT")
                for mg in range(8):
                    if mg < 2:
                        w1_s = w1_pre[mg]
                    else:
                        w1_s = w1pool.tile([128, KT, 512], BF16, tag="wA",
                                           name="w1_s")
                        nc.sync.dma_start(
                            w1_s,
                            w1_d[:, mg // 2, :,
                                 512 * (mg % 2):512 * (mg % 2 + 1)])
                    for mm in range(4):
                        m = mg * 4 + mm
                        pm = ps_mm.tile([128, N], F32, tag="mm")
                        f1_src = yp if last_blk else x_s
                        for k in range(KT):
                            nc.tensor.matmul(pm,
                                             w1_s[:, k, 128 * mm:128 * (mm + 1)],
                                             f1_src[:, k, :],
                                             start=(k == 0), stop=(k == KT - 1))
                        if last_blk:
                            tg = work.tile([128, N], BF16, tag="fz", bufs=2,
                                           name="tg")
                            nc.vector.tensor_mul(tg, pm, r1_s)
                            nc.scalar.activation(hT[:, m, :], tg, AF.Gelu,
                                                 bias=b1_s[:, m:m + 1])
                        else:
                            nc.scalar.activation(hT[:, m, :], pm, AF.Gelu,
                                                 bias=b1_s[:, m:m + 1])
                if last_blk:
                    # residual x needed only from FFN2 onward
                    x_mults()

                # ---- FFN2 + residual + bf16 transpose -------------------
                # w2 stream runs one half-slice ahead of the consuming
                # matmuls (bufs=2: one in use, one loading).
                def load_w2(m, kh):
                    w2_s = w2pool.tile([128, 16, 128], BF16, tag="w2s",
                                       name="w2_s")
                    nc.sync.dma_start(w2_s,
                                      w2_d[:, m, 16 * kh:16 * (kh + 1), :])
                    return w2_s

                z_rm = [rm.tile([128, D], F32, tag="z_rm", bufs=ni,
                                name="z_rm")
                        for _ in range(ni)]
                w2_nxt = load_w2(0, 0)
                for m in range(MT):
                    pm = ps_mm.tile([128, N], F32, tag="mm")
                    for kh in range(2):
                        w2_s = w2_nxt
                        if not (m == MT - 1 and kh == 1):
                            w2_nxt = load_w2(m + kh, (kh + 1) % 2)
                        for kk in range(16):
                            k = 16 * kh + kk
                            nc.tensor.matmul(pm, w2_s[:, kk, :], hT[:, k, :],
                                             start=(k == 0),
                                             stop=(k == FMT - 1))
                    fz = work.tile([128, N], BF16, tag="fz", bufs=2)
                    nc.scalar.activation(fz, pm, AF.Identity,
                                         bias=b2_s[:, m:m + 1])
                    if m == 1 and blk + 1 == nb:
                        # pre-load the Sqrt ACT table now (hidden under
                        # FFN2) so the tail's LN2 chain doesn't pay it;
                        # reading fz pins it after the m=1 drain so the
                        # scheduler can't hoist it ahead of the gelus
                        dum = st.tile([1, 1], F32, tag="dum")
                        nc.scalar.activation(dum, fz[0:1, 0:1], AF.Sqrt)
                    zt = work.tile([128, N], BF16, tag="zt")
                    nc.vector.tensor_add(zt, x_s[:, m, :], fz)
                    ptr = ps_tr.tile([128, ni * 128], BF16)
                    for i in range(ni):
                        nc.tensor.transpose(ptr[:, 128 * i:128 * (i + 1)],
                                            zt[:, 128 * i:128 * (i + 1)], ident)
                    for i in range(ni):
                        nc.scalar.copy(z_rm[i][:, 128 * m:128 * (m + 1)],
                                       ptr[:, 128 * i:128 * (i + 1)])

                pending_ln2 = (z_rm, r0)
                if blk + 1 < nb:
                    t_cur = t_nxt
            emit_ln2(*pending_ln2, tail=True)

    nc.compile()
    return nc


def host_prepare(inputs):
    """Fold parameters and lay out weights for the device (all O(params))."""
    f64 = {k: np.asarray(inputs[k], dtype=np.float64)
           for k in ("dom_movie", "w_q", "w_k", "w_v", "b_q", "w_o", "b_o",
                     "b_v")}
    qs = (f64["dom_movie"] @ f64["w_q"].T + f64["b_q"]) / np.sqrt(HD)  # (1, D)
    qh = qs.reshape(H, HD)
    A = np.einsum("hd,hdD->hD", qh, f64["w_k"].reshape(H, HD, D))  # (H, D)
    bod2 = f64["b_o"] + f64["dom_movie"][0] + f64["w_o"] @ f64["b_v"]  # (D,)

    # LN centering folded into the output projection: C = I - 11^T/D
    woC = f64["w_o"] - f64["w_o"].mean(axis=0, keepdims=True)   # C @ w_o
    M = woC @ f64["w_v"]                                        # C w_o w_v
    bodC = bod2 - bod2.mean()                                   # C @ bod2

    E = np.zeros((H, MT, 128), np.float32)
    for m in range(MT):
        for p in range(128):
            E[2 * m + p // 64, m, p] = 1.0

    w1 = np.asarray(inputs["w1"], np.float32)
    w2 = np.asarray(inputs["w2"], np.float32)

    def fm_weight(wT):  # wT (d_in, d_out) -> [128, d_in/128, d_out]
        return np.ascontiguousarray(
            wT.reshape(-1, 128, wT.shape[1]).transpose(1, 0, 2)).astype(NPBF16)

    prep = {
        "wvT": fm_weight(np.asarray(f64["w_v"], np.float32).T),
        "wmT": fm_weight(np.asarray(M, np.float32).T),
        "woT": fm_weight(np.asarray(woC, np.float32).T),
        "w1P": np.ascontiguousarray(
            w1.T.reshape(KT, 128, NMG, D).transpose(1, 2, 0, 3)).astype(NPBF16),
        "w2P": np.ascontiguousarray(
            w2.T.reshape(FMT, 128, MT, 128).transpose(1, 2, 0, 3)).astype(NPBF16),
        "AT": np.ascontiguousarray(
            A.T.reshape(KT, 128, H).transpose(1, 0, 2)).astype(NPBF16),
        "E": E.astype(NPBF16),
        "bodC": np.ascontiguousarray(
            bodC.reshape(MT, 128).T).astype(np.float32),
        "b1p": np.ascontiguousarray(
            np.asarray(inputs["b1"], np.float64).reshape(FMT, 128).T
        ).astype(np.float32),
        "b2p": np.ascontiguousarray(
            np.asarray(inputs["b2"], np.float64).reshape(MT, 128).T
        ).astype(np.float32),
    }
    trivial_ln1 = bool(np.all(np.asarray(inputs["ln1_g"]) == 1.0)
                       and np.all(np.asarray(inputs["ln1_b"]) == 0.0))
    trivial_ln2 = bool(np.all(np.asarray(inputs["ln2_g"]) == 1.0)
                       and np.all(np.asarray(inputs["ln2_b"]) == 0.0))
    if not trivial_ln1:
        prep["g1p"] = np.ascontiguousarray(
            np.asarray(inputs["ln1_g"], np.float64).reshape(MT, 128).T
        ).astype(np.float32)
        prep["c1p"] = np.ascontiguousarray(
            np.asarray(inputs["ln1_b"], np.float64).reshape(MT, 128).T
        ).astype(np.float32)
    if not trivial_ln2:
        prep["g2"] = np.asarray(inputs["ln2_g"], np.float32)
        prep["c2"] = np.asarray(inputs["ln2_b"], np.float32)
    return prep, trivial_ln1, trivial_ln2


_PROGRAM_CACHE = {}


def _get_program(b_core, r_blk, t1, t2):
    key = (b_core, r_blk, t1, t2)
    if key not in _PROGRAM_CACHE:
        _PROGRAM_CACHE[key] = build_program(b_core, r_blk, t1, t2)
    return _PROGRAM_CACHE[key]


def kernel(h_u_cross, h_u_target, dom_movie, w_q, w_k, w_v, b_q, b_k, b_v,
           w_o, b_o, ln1_g, ln1_b, w1, b1, w2, b2, ln2_g, ln2_b,
           trace=False, r_blk=512, **run_kwargs):
    inputs = dict(h_u_cross=h_u_cross, h_u_target=h_u_target,
                  dom_movie=dom_movie, w_q=w_q, w_k=w_k, w_v=w_v, b_q=b_q,
                  b_k=b_k, b_v=b_v, w_o=w_o, b_o=b_o, ln1_g=ln1_g,
                  ln1_b=ln1_b, w1=w1, b1=b1, w2=w2, b2=b2, ln2_g=ln2_g,
                  ln2_b=ln2_b)
    prep, t1, t2 = host_prepare(inputs)
    nc = _get_program(B_CORE, r_blk, t1, t2)

    xc = np.asarray(h_u_cross, np.float32)
    xt = np.asarray(h_u_target, np.float32)
    xtb = np.ascontiguousarray(xt.astype(NPBF16))
    db = np.ascontiguousarray((xc - xt).astype(NPBF16))
    in_maps = []
    for c in range(N_CORES):
        m = dict(prep)
        m["xtb"] = xtb[c * B_CORE:(c + 1) * B_CORE]
        m["db"] = db[c * B_CORE:(c + 1) * B_CORE]
        in_maps.append(m)

    res = run_bass_kernel_spmd(nc, in_maps, core_ids=list(range(N_CORES)),
                               trace=trace, **run_kwargs)
    out = np.concatenate([res.results[c]["out"] for c in range(N_CORES)], axis=0)
    kernel.last_results = res
    return out.astype(np.float32)


# revision 33
# speedup vs baseline: 1.0277x; 1.0081x over previous
"""Trainium2 Bass kernel for AttentionConditionGenerator.

Reference computation (per row b of B=16384):
    kv = [h_u_cross[b], h_u_target[b]]            # (2, 1024)
    q  = dom_movie @ w_q.T + b_q                  # fixed across rows
    scores = (q/8) . k[s],  attn = softmax_2(scores)
    ctx = attn0*v0 + attn1*v1 ; y = ctx @ w_o.T + b_o
    x = LN1(dom_movie + y); h = gelu(x @ w1.T + b1)
    out = LN2(x + h @ w2.T + b2)

Algebraic folding (host, fp64, exact):
  - q row-independent -> scores fold to A @ d with d = xc - xt;
    attn0 = sigmoid(A @ d) (b_k cancels in the 2-way softmax).
  - ctx = v_t + attn0 * v_d, v_t = w_v@xt + b_v, v_d = w_v@d.
  - LN1 centering folded into the weights: with C = I - 11^T/D,
        y' = C@y = (C@w_o@w_v)@xt + (C@w_o)@(attn0*(w_v@d)) + C@bod2
    so y' arrives centered and LN1 reduces to a per-row scale:
        x = y' * rsqrt(mean(y'^2) + eps)
    x is exactly column-centered too, so LN2 sees a centered residual.

Device mapping: batch split over 8 cores (2048 rows each). Activations stay
feature-major (features on partitions) end-to-end; LN1 runs feature-major
(variance via ones-vector matmuls + a 1-row broadcast matmul), LN2 runs
row-major after cheap bf16 transposes of the final residual. All matmuls are
bf16 with fp32 PSUM accumulation.
"""

import numpy as np
import ml_dtypes

try:
    import concourse.bass as bass
except ImportError:  # pragma: no cover - path setup for fresh environments
    import sys

    for _p in ("/opt/trn_rl_repo", "/root/.axon_site/_ro/trn_rl_repo"):
        if _p not in sys.path:
            sys.path.insert(0, _p)
    import concourse.bass as bass

import concourse.mybir as mybir
import concourse.tile as tile
from concourse import bacc
from concourse.bass_utils import run_bass_kernel_spmd
from concourse.masks import make_identity

F32 = mybir.dt.float32
BF16 = mybir.dt.bfloat16
NPBF16 = ml_dtypes.bfloat16

D = 1024
H = 16
HD = 64
FFN = 4096
EPS = 1e-5
N_CORES = 8
B_TOTAL = 16384
B_CORE = B_TOTAL // N_CORES  # 2048

KT = D // 128  # 8 feature k-tiles
MT = D // 128  # 8 output m-tiles
FMT = FFN // 128  # 32 FFN m-tiles
NMG = 4  # host-side FFN1 m-group axis (1024 cols each)

AF = mybir.ActivationFunctionType


def build_program(b_core, r_blk, trivial_ln1, trivial_ln2):
    """Build and compile the per-core Bass program."""
    nb = b_core // r_blk  # row blocks
    ni = r_blk // 128  # 128-row subtiles per block
    N = r_blk  # matmul moving (free) dim

    nc = bacc.Bacc("TRN2", target_bir_lowering=False)

    # ---- DRAM I/O ------------------------------------------------------
    xtb_d = nc.dram_tensor("xtb", [b_core, D], BF16, kind="ExternalInput")
    db_d = nc.dram_tensor("db", [b_core, D], BF16, kind="ExternalInput")
    wv_d = nc.dram_tensor("wvT", [128, KT, D], BF16, kind="ExternalInput")
    wm_d = nc.dram_tensor("wmT", [128, KT, D], BF16, kind="ExternalInput")
    wo_d = nc.dram_tensor("woT", [128, KT, D], BF16, kind="ExternalInput")
    w1_d = nc.dram_tensor("w1P", [128, NMG, KT, D], BF16, kind="ExternalInput")
    w2_d = nc.dram_tensor("w2P", [128, MT, FMT, 128], BF16, kind="ExternalInput")
    at_d = nc.dram_tensor("AT", [128, KT, H], BF16, kind="ExternalInput")
    e_d = nc.dram_tensor("E", [H, MT, 128], BF16, kind="ExternalInput")
    bod_d = nc.dram_tensor("bodC", [128, MT], F32, kind="ExternalInput")
    b1_d = nc.dram_tensor("b1p", [128, FMT], F32, kind="ExternalInput")
    b2_d = nc.dram_tensor("b2p", [128, MT], F32, kind="ExternalInput")
    if not trivial_ln1:
        g1_d = nc.dram_tensor("g1p", [128, MT], F32, kind="ExternalInput")
        c1_d = nc.dram_tensor("c1p", [128, MT], F32, kind="ExternalInput")
    if not trivial_ln2:
        g2_d = nc.dram_tensor("g2", [D], F32, kind="ExternalInput")
        c2_d = nc.dram_tensor("c2", [D], F32, kind="ExternalInput")
    out_d = nc.dram_tensor("out", [b_core, D], F32, kind="ExternalOutput")

    def bcast_ap(dram):
        # [D] dram vector -> [128, D] partition-broadcast access pattern
        return bass.AP(tensor=dram, offset=0, ap=[[0, 128], [1, D]])

    with tile.TileContext(nc) as tc:
        with (
            tc.tile_pool(name="consts", bufs=1) as consts,
            tc.tile_pool(name="wres", bufs=1) as wres,
            tc.tile_pool(name="w1s", bufs=2) as w1pool,
            tc.tile_pool(name="w2s", bufs=2) as w2pool,
            tc.tile_pool(name="fm", bufs=1) as fm,
            tc.tile_pool(name="act", bufs=1) as am,
            tc.tile_pool(name="work", bufs=3) as work,
            tc.tile_pool(name="stats", bufs=2) as st,
            tc.tile_pool(name="rm", bufs=2) as rm,
            tc.tile_pool(name="ps_mm", bufs=3, space="PSUM") as ps_mm,
            tc.tile_pool(name="ps_ab", bufs=2, space="PSUM") as ps_ab,
            tc.tile_pool(name="ps_misc", bufs=2, space="PSUM") as ps_misc,
            tc.tile_pool(name="ps_tr", bufs=1, space="PSUM") as ps_tr,
        ):
            # ---- constants (at_s first: scores need it immediately;
            # the rest are loaded after the cold input transposes) -----
            at_s = consts.tile([128, KT, H], BF16)
            nc.sync.dma_start(at_s, at_d[:, :, :])
            ident = consts.tile([128, 128], BF16)
            e_s = consts.tile([H, MT, 128], BF16)
            bod_s = consts.tile([128, MT], F32)
            b1_s = consts.tile([128, FMT], F32)
            b2_s = consts.tile([128, MT], F32)
            ones_col = consts.tile([128, 1], BF16)
            nc.vector.memset(ones_col, 1.0)
            ones_f8 = consts.tile([128, 2, 64], mybir.dt.float8e4)
            nc.vector.memset(ones_f8, 1.0)
            ones_row = consts.tile([1, 128], BF16)
            nc.vector.memset(ones_row, 1.0)
            eps_s = consts.tile([128, 1], F32)
            nc.vector.memset(eps_s, EPS)
            eps1 = consts.tile([1, 1], F32)
            nc.vector.memset(eps1, EPS)
            g1_s = c1_s = g2_s = c2_s = None
            if not trivial_ln1:
                g1_s = consts.tile([128, MT], F32)
                nc.sync.dma_start(g1_s, g1_d[:, :])
                c1_s = consts.tile([128, MT], F32)
                nc.sync.dma_start(c1_s, c1_d[:, :])
            if not trivial_ln2:
                g2_s = consts.tile([128, D], F32)
                nc.gpsimd.dma_start(g2_s, bcast_ap(g2_d))
                c2_s = consts.tile([128, D], F32)
                nc.gpsimd.dma_start(c2_s, bcast_ap(c2_d))

            def layernorm_rm(y_i, out_tile, g_s, c_s, last=False):
                """Row-major LayerNorm of y_i [128, D] f32 -> out_tile.
                last=True puts the normalize on DVE (the Pool impl is
                ~2x slower, which only matters on the kernel tail)."""
                stt = st.tile([128, 2, 6], F32, tag="bnst")
                nc.vector.bn_stats(stt[:, 0, :], y_i[:, 0:512])
                nc.vector.bn_stats(stt[:, 1, :], y_i[:, 512:1024])
                mv = st.tile([128, 2], F32, tag="bnmv")
                nc.vector.bn_aggr(mv, stt)
                std = st.tile([128, 1], F32, tag="bnstd")
                nc.scalar.activation(std, mv[:, 1:2], AF.Sqrt, bias=eps_s)
                rstd = st.tile([128, 1], F32, tag="bnrstd")
                nc.vector.reciprocal(rstd, std)
                eng = nc.vector if last else nc.gpsimd
                if g_s is None:
                    eng.tensor_scalar(
                        out_tile, y_i, mv[:, 0:1], rstd,
                        op0=mybir.AluOpType.subtract, op1=mybir.AluOpType.mult)
                else:
                    xn = st.tile([128, D], F32, tag="bnxn")
                    nc.gpsimd.tensor_scalar(
                        xn, y_i, mv[:, 0:1], rstd,
                        op0=mybir.AluOpType.subtract, op1=mybir.AluOpType.mult)
                    nc.gpsimd.tensor_mul(out_tile, xn, g_s)
                    nc.gpsimd.tensor_add(out_tile, out_tile, c_s)

            def load_inputs(blk):
                """DMA-transpose-load block inputs (dT first: scores need
                it). dT is split into k-halves so consumers of early
                k-tiles need not wait for the full transfer."""
                r0 = blk * r_blk
                dTa = fm.tile([128, KT // 2, N], BF16, tag="dTa", bufs=2)
                dTb = fm.tile([128, KT // 2, N], BF16, tag="dTb", bufs=2)
                xtT = fm.tile([128, KT, N], BF16, tag="xtT", bufs=2)
                nc.sync.dma_start(dTa, db_d[r0:r0 + N, 0:512], transpose=True)
                nc.sync.dma_start(dTb, db_d[r0:r0 + N, 512:1024],
                                  transpose=True)
                nc.sync.dma_start(xtT[:, 0:4, :], xtb_d[r0:r0 + N, 0:512],
                                  transpose=True)
                nc.sync.dma_start(xtT[:, 4:8, :], xtb_d[r0:r0 + N, 512:1024],
                                  transpose=True)
                return xtT, (dTa, dTb)

            def dk(dT, k):
                return dT[k // 4][:, k % 4, :]


            def scores_mm(dT):
                """Raw attention scores A @ d -> PSUM (sigmoid deferred)."""
                psc = ps_misc.tile([H, N], F32, tag="misc")
                for k in range(KT):
                    nc.tensor.matmul(psc, at_s[:, k, :], dk(dT, k),
                                     start=(k == 0), stop=(k == KT - 1))
                return psc

            def scores_act(psc):
                attn0 = st.tile([H, N], BF16, tag="attn0", bufs=2)
                nc.scalar.activation(attn0, psc, AF.Sigmoid)
                return attn0

            def attn_u(dT, m):
                pu = ps_mm.tile([128, N], F32, tag="mm", name="pu")
                for k in range(KT):
                    nc.tensor.matmul(pu,
                                     wv_s[:, k, 128 * m:128 * (m + 1)],
                                     dk(dT, k),
                                     start=(k == 0), stop=(k == KT - 1))
                return pu

            def attn_t(t_tile, dT, attn0, m, pu=None):
                if pu is None:
                    pu = attn_u(dT, m)
                pab = ps_ab.tile([128, N], F32, name="pab")
                nc.tensor.matmul(pab, e_s[:, m, :], attn0,
                                 start=True, stop=True)
                # the BIR verifier rejects TensorTensor with two PSUM
                # operands: drain the broadcast through ACT first
                ab_s = work.tile([128, N], BF16, tag="ab", bufs=2)
                nc.scalar.copy(ab_s, pab)
                nc.vector.tensor_mul(t_tile[:, m, :], pu, ab_s)

            # ---- cold start: block 0 inputs + resident weights, ordered
            # by first use (scores->dT, u->wv, y->xtT/wm/wo) --------------
            dT0a = fm.tile([128, KT // 2, N], BF16, tag="dTa", bufs=2)
            dT0b = fm.tile([128, KT // 2, N], BF16, tag="dTb", bufs=2)
            dT0 = (dT0a, dT0b)
            xtT0 = fm.tile([128, KT, N], BF16, tag="xtT", bufs=2)
            wv_s = wres.tile([128, KT, D], BF16)
            wm_s = wres.tile([128, KT, D], BF16)
            wo_s = wres.tile([128, KT, D], BF16)
            # first input: dT halves split across the two DMA queues so
            # HWDGE descriptor dispatch (625ns each) isn't the critical
            # path into the first score/u matmuls
            nc.sync.dma_start(dT0a, db_d[0:N, 0:512], transpose=True)
            nc.sync.dma_start(dT0b, db_d[0:N, 512:1024], transpose=True)
            for mh in range(2):
                nc.sync.dma_start(wv_s[:, :, 512 * mh:512 * (mh + 1)],
                                  wv_d[:, :, 512 * mh:512 * (mh + 1)])
            nc.sync.dma_start(xtT0[:, 0:4, :], xtb_d[0:N, 0:512],
                              transpose=True)
            make_identity(nc, ident)
            nc.gpsimd.dma_start(e_s, e_d[:, :, :])
            nc.sync.dma_start(xtT0[:, 4:8, :], xtb_d[0:N, 512:1024],
                              transpose=True)
            nc.sync.dma_start(bod_s, bod_d[:, :])
            for q in range(4):
                for w_s, w_d_ in ((wm_s, wm_d), (wo_s, wo_d)):
                    nc.sync.dma_start(w_s[:, :, 256 * q:256 * (q + 1)],
                                      w_d_[:, :, 256 * q:256 * (q + 1)])
            nc.sync.dma_start(b1_s, b1_d[:, :])
            nc.sync.dma_start(b2_s, b2_d[:, :])
            nxt = (xtT0, dT0)
            nxt_at = scores_act(scores_mm(dT0))
            t_cur = am.tile([128, MT, N], BF16, tag="t", bufs=2, name="t_cur")
            pending_ln2 = None

            def emit_ln2(z_rm, r0, tail):
                """Row-major LN2 + store for one block's z_rm tiles.
                tail=True phase-batches the chains (shorter critical path
                at the very end of the kernel); otherwise per-i chains
                with the normalize on Pool (DVE stays free for the
                current block's attention)."""
                g_s = None if trivial_ln2 else g2_s
                c_s = None if trivial_ln2 else c2_s
                if not tail:
                    for i in range(ni):
                        layernorm_rm(z_rm[i], z_rm[i], g_s, c_s)
                        nc.gpsimd.dma_start(
                            out_d[r0 + 128 * i:r0 + 128 * (i + 1), :], z_rm[i])
                    return
                stts, mvs, stds, rstds = [], [], [], []
                for i in range(ni):
                    stt = st.tile([128, 2, 6], F32, tag="bnstT", bufs=ni,
                                  name="stt")
                    nc.vector.bn_stats(stt[:, 0, :], z_rm[i][:, 0:512])
                    nc.vector.bn_stats(stt[:, 1, :], z_rm[i][:, 512:1024])
                    stts.append(stt)
                for i in range(ni):
                    mv = st.tile([128, 2], F32, tag="bnmvT", bufs=ni, name="mv")
                    nc.vector.bn_aggr(mv, stts[i])
                    mvs.append(mv)
                for i in range(ni):
                    std = st.tile([128, 1], F32, tag="bnstdT", bufs=ni,
                                  name="std_i")
                    nc.scalar.activation(std, mvs[i][:, 1:2], AF.Sqrt,
                                         bias=eps_s)
                    stds.append(std)
                for i in range(ni):
                    rstd = st.tile([128, 1], F32, tag="bnrstdT", bufs=ni,
                                   name="rstd_i")
                    nc.vector.reciprocal(rstd, stds[i])
                    rstds.append(rstd)
                for i in range(ni):
                    nc.vector.tensor_scalar(
                        z_rm[i], z_rm[i], mvs[i][:, 0:1], rstds[i],
                        op0=mybir.AluOpType.subtract,
                        op1=mybir.AluOpType.mult)
                    if g_s is not None:
                        nc.vector.tensor_mul(z_rm[i], z_rm[i], g_s)
                        nc.vector.tensor_add(z_rm[i], z_rm[i], c_s)
                    # alternate queues so the final stores overlap; SP is
                    # idle here (no more weight traffic)
                    q = nc.sync if i % 2 == 0 else nc.gpsimd
                    q.dma_start(
                        out_d[r0 + 128 * i:r0 + 128 * (i + 1), :], z_rm[i])

            for blk in range(nb):
                r0 = blk * r_blk
                xtT, dT = nxt
                attn0 = nxt_at

                # next block's inputs: the sync DMA queue is empty here
                if blk + 1 < nb:
                    nxt = load_inputs(blk + 1)

                # ---- attention: t = attn0 * (w_v @ d) -------------------
                # (m=0,1 of this block were pulled into the previous
                # block's LN1 window as PE filler)
                for m in range(0 if blk == 0 else 2, MT):
                    attn_t(t_cur, dT, attn0, m)

                # previous block's LN2 + store, emitted here so its DVE
                # chains fill this block's y-group window instead of
                # racing this block's t-mults
                if pending_ln2 is not None:
                    emit_ln2(*pending_ln2, tail=False)
                    pending_ln2 = None

                # ---- y' = (C w_o w_v)@xt + (C w_o)@t + bodC  (centered) -
                yp = am.tile([128, MT, N], BF16, tag="yp")
                ss = ps_misc.tile([64, N], F32, tag="misc")
                for m in range(MT):
                    py = ps_mm.tile([128, N], F32, tag="mm")
                    for k in range(KT):
                        nc.tensor.matmul(py,
                                         wm_s[:, k, 128 * m:128 * (m + 1)],
                                         xtT[:, k, :],
                                         start=(k == 0), stop=False)
                    for k in range(KT):
                        nc.tensor.matmul(py,
                                         wo_s[:, k, 128 * m:128 * (m + 1)],
                                         t_cur[:, k, :],
                                         start=False, stop=(k == KT - 1))
                    nc.scalar.activation(yp[:, m, :], py, AF.Identity,
                                         bias=bod_s[:, m:m + 1])
                    if m % 2 == 0:
                        y2p = work.tile([128, 2, N], mybir.dt.float8e4,
                                        tag="y2", bufs=2, name="y2p")
                    nc.scalar.activation(y2p[:, m % 2, :], py, AF.Square,
                                         bias=bod_s[:, m:m + 1])
                    if m % 2 == 1:
                        # fp8 DoubleRow: both 128-partition halves of the
                        # pair sum into ss in N/2 cycles (squares only
                        # need ~0.2% precision for the variance)
                        nc.tensor.matmul(ss, ones_f8, y2p,
                                         start=(m == 1), stop=(m == MT - 1),
                                         perf_mode=mybir.MatmulPerfMode.DoubleRow)

                # prefetch the first two FFN1 weight slices now
                w1_pre = []
                for mg in range(2):
                    w1_s = w1pool.tile([128, KT, 512], BF16, tag="wA",
                                       name="w1_s")
                    nc.sync.dma_start(
                        w1_s,
                        w1_d[:, mg // 2, :, 512 * (mg % 2):512 * (mg % 2 + 1)])
                    w1_pre.append(w1_s)

                # ---- LN1 scale: x = y' * rsqrt(mean(y'^2)+eps) ----------
                # The sqrt->recip->bcast chain has no PE work of its own;
                # next block's scores + attention m=0,1 fill the bubble.
                std = st.tile([1, N], F32, tag="std")
                nc.scalar.activation(std, ss[0:1, :], AF.Sqrt, bias=eps1,
                                     scale=1.0 / D)
                rstd_bf = st.tile([1, N], BF16, tag="rstdb")
                with nc.allow_low_precision(
                        reason="bf16 rstd: 0.1%% scale noise, LN2 renormalizes"):
                    nc.vector.reciprocal(rstd_bf, std)
                if blk + 1 < nb:
                    nxt_at = scores_act(scores_mm(nxt[1]))
                    t_nxt = am.tile([128, MT, N], BF16, tag="t", bufs=2,
                                    name="t_nxt")
                    pu0 = attn_u(nxt[1], 0)
                    pu1 = attn_u(nxt[1], 1)
                    attn_t(t_nxt, nxt[1], nxt_at, 0, pu=pu0)
                    attn_t(t_nxt, nxt[1], nxt_at, 1, pu=pu1)
                pr1 = ps_misc.tile([128, N], F32, tag="misc")
                nc.tensor.matmul(pr1, ones_row, rstd_bf, start=True, stop=True)
                r1_s = work.tile([128, N], BF16, tag="r1", bufs=2)
                nc.scalar.copy(r1_s, pr1)
                x_s = am.tile([128, MT, N], BF16, tag="x")

                def x_mults():
                    for m in range(MT):
                        if trivial_ln1:
                            # all-bf16 SBUF operands: 2x DVE mode
                            nc.vector.tensor_mul(x_s[:, m, :], yp[:, m, :],
                                                 r1_s)
                        else:
                            xm = work.tile([128, N], F32, tag="xm")
                            nc.vector.tensor_mul(xm, yp[:, m, :], r1_s)
                            nc.vector.tensor_scalar(
                                x_s[:, m, :], xm, g1_s[:, m:m + 1],
                                c1_s[:, m:m + 1],
                                op0=mybir.AluOpType.mult,
                                op1=mybir.AluOpType.add)

                if blk + 1 < nb or not trivial_ln1:
                    x_mults()

                # ---- FFN1: h = gelu(w1 @ x + b1) ------------------------
                # Last block: no next-block filler exists for the LN1
                # chain, so run the matmuls on raw y' and fold the
                # per-column rstd scale in after the matmul (exact:
                # w1 @ (y'*r) = (w1 @ y') * r). PE never waits on rstd.
                last_blk = blk == nb - 1 and trivial_ln1
                hT = am.tile([128, FMT, N], BF16, tag="hT")
                for mg in range(8):
                    if mg < 2:
                        w1_s = w1_pre[mg]
                    else:
                        w1_s = w1pool.tile([128, KT, 512], BF16, tag="wA",
                                           name="w1_s")
                        nc.sync.dma_start(
                            w1_s,
                            w1_d[:, mg // 2, :,
                                 512 * (mg % 2):512 * (mg % 2 + 1)])
                    for mm in range(4):
                        m = mg * 4 + mm
                        pm = ps_mm.tile([128, N], F32, tag="mm")
                        f1_src = yp if last_blk else x_s
                        for k in range(KT):
                            nc.tensor.matmul(pm,
                                             w1_s[:, k, 128 * mm:128 * (mm + 1)],
                                             f1_src[:, k, :],
                                             start=(k == 0), stop=(k == KT - 1))
                        if last_blk:
                            tg = work.tile([128, N], BF16, tag="fz", bufs=2,
                                           name="tg")
                            nc.vector.tensor_mul(tg, pm, r1_s)
                            nc.scalar.activation(hT[:, m, :], tg, AF.Gelu,
                                                 bias=b1_s[:, m:m + 1])
                        else:
                            nc.scalar.activation(hT[:, m, :], pm, AF.Gelu,
                                                 bias=b1_s[:, m:m + 1])
                if last_blk:
                    # residual x needed only from FFN2 onward
                    x_mults()

                # ---- FFN2 + residual + bf16 transpose -------------------
                # w2 stream runs one half-slice ahead of the consuming
                # matmuls (bufs=2: one in use, one loading).
                def load_w2(m, kh):
                    w2_s = w2pool.tile([128, 16, 128], BF16, tag="w2s",
                                       name="w2_s")
                    nc.sync.dma_start(w2_s,
                                      w2_d[:, m, 16 * kh:16 * (kh + 1), :])
                    return w2_s

                z_rm = [rm.tile([128, D], F32, tag="z_rm", bufs=ni,
                                name="z_rm")
                        for _ in range(ni)]
                w2_nxt = load_w2(0, 0)
                for m in range(MT):
                    pm = ps_mm.tile([128, N], F32, tag="mm")
                    for kh in range(2):
                        w2_s = w2_nxt
                        if not (m == MT - 1 and kh == 1):
                            w2_nxt = load_w2(m + kh, (kh + 1) % 2)
                        for kk in range(16):
                            k = 16 * kh + kk
                            nc.tensor.matmul(pm, w2_s[:, kk, :], hT[:, k, :],
                                             start=(k == 0),
                                             stop=(k == FMT - 1))
                    fz = work.tile([128, N], BF16, tag="fz", bufs=2)
                    nc.scalar.activation(fz, pm, AF.Identity,
                                         bias=b2_s[:, m:m + 1])
                    if m == 1 and blk + 1 == nb:
                        # pre-load the Sqrt ACT table now (hidden under
                        # FFN2) so the tail's LN2 chain doesn't pay it;
                        # reading fz pins it after the m=1 drain so the
                        # scheduler can't hoist it ahead of the gelus
                        dum = st.tile([1, 1], F32, tag="dum")
                        nc.scalar.activation(dum, fz[0:1, 0:1], AF.Sqrt)
                    zt = work.tile([128, N], BF16, tag="zt")
                    nc.vector.tensor_add(zt, x_s[:, m, :], fz)
                    ptr = ps_tr.tile([128, ni * 128], BF16)
                    for i in range(ni):
                        nc.tensor.transpose(ptr[:, 128 * i:128 * (i + 1)],
                                            zt[:, 128 * i:128 * (i + 1)], ident)
                    for i in range(ni):
                        nc.scalar.copy(z_rm[i][:, 128 * m:128 * (m + 1)],
                                       ptr[:, 128 * i:128 * (i + 1)])

                pending_ln2 = (z_rm, r0)
                if blk + 1 < nb:
                    t_cur = t_nxt
            emit_ln2(*pending_ln2, tail=True)

    nc.compile()
    return nc


def host_prepare(inputs):
    """Fold parameters and lay out weights for the device (all O(params))."""
    f64 = {k: np.asarray(inputs[k], dtype=np.float64)
           for k in ("dom_movie", "w_q", "w_k", "w_v", "b_q", "w_o", "b_o",
                     "b_v")}
    qs = (f64["dom_movie"] @ f64["w_q"].T + f64["b_q"]) / np.sqrt(HD)  # (1, D)
    qh = qs.reshape(H, HD)
    A = np.einsum("hd,hdD->hD", qh, f64["w_k"].reshape(H, HD, D))  # (H, D)
    bod2 = f64["b_o"] + f64["dom_movie"][0] + f64["w_o"] @ f64["b_v"]  # (D,)

    # LN centering folded into the output projection: C = I - 11^T/D
    woC = f64["w_o"] - f64["w_o"].mean(axis=0, keepdims=True)   # C @ w_o
    M = woC @ f64["w_v"]                                        # C w_o w_v
    bodC = bod2 - bod2.mean()                                   # C @ bod2

    E = np.zeros((H, MT, 128), np.float32)
    for m in range(MT):
        for p in range(128):
            E[2 * m + p // 64, m, p] = 1.0

    w1 = np.asarray(inputs["w1"], np.float32)
    w2 = np.asarray(inputs["w2"], np.float32)

    def fm_weight(wT):  # wT (d_in, d_out) -> [128, d_in/128, d_out]
        return np.ascontiguousarray(
            wT.reshape(-1, 128, wT.shape[1]).transpose(1, 0, 2)).astype(NPBF16)

    prep = {
        "wvT": fm_weight(np.asarray(f64["w_v"], np.float32).T),
        "wmT": fm_weight(np.asarray(M, np.float32).T),
        "woT": fm_weight(np.asarray(woC, np.float32).T),
        "w1P": np.ascontiguousarray(
            w1.T.reshape(KT, 128, NMG, D).transpose(1, 2, 0, 3)).astype(NPBF16),
        "w2P": np.ascontiguousarray(
            w2.T.reshape(FMT, 128, MT, 128).transpose(1, 2, 0, 3)).astype(NPBF16),
        "AT": np.ascontiguousarray(
            A.T.reshape(KT, 128, H).transpose(1, 0, 2)).astype(NPBF16),
        "E": E.astype(NPBF16),
        "bodC": np.ascontiguousarray(
            bodC.reshape(MT, 128).T).astype(np.float32),
        "b1p": np.ascontiguousarray(
            np.asarray(inputs["b1"], np.float64).reshape(FMT, 128).T
        ).astype(np.float32),
        "b2p": np.ascontiguousarray(
            np.asarray(inputs["b2"], np.float64).reshape(MT, 128).T
        ).astype(np.float32),
    }
    trivial_ln1 = bool(np.all(np.asarray(inputs["ln1_g"]) == 1.0)
                       and np.all(np.asarray(inputs["ln1_b"]) == 0.0))
    trivial_ln2 = bool(np.all(np.asarray(inputs["ln2_g"]) == 1.0)
                       and np.all(np.asarray(inputs["ln2_b"]) == 0.0))
    if not trivial_ln1:
        prep["g1p"] = np.ascontiguousarray(
            np.asarray(inputs["ln1_g"], np.float64).reshape(MT, 128).T
        ).astype(np.float32)
        prep["c1p"] = np.ascontiguousarray(
            np.asarray(inputs["ln1_b"], np.float64).reshape(MT, 128).T
        ).astype(np.float32)
    if not trivial_ln2:
        prep["g2"] = np.asarray(inputs["ln2_g"], np.float32)
        prep["c2"] = np.asarray(inputs["ln2_b"], np.float32)
    return prep, trivial_ln1, trivial_ln2


_PROGRAM_CACHE = {}


def _get_program(b_core, r_blk, t1, t2):
    key = (b_core, r_blk, t1, t2)
    if key not in _PROGRAM_CACHE:
        _PROGRAM_CACHE[key] = build_program(b_core, r_blk, t1, t2)
    return _PROGRAM_CACHE[key]


def kernel(h_u_cross, h_u_target, dom_movie, w_q, w_k, w_v, b_q, b_k, b_v,
           w_o, b_o, ln1_g, ln1_b, w1, b1, w2, b2, ln2_g, ln2_b,
           trace=False, r_blk=512, **run_kwargs):
    inputs = dict(h_u_cross=h_u_cross, h_u_target=h_u_target,
                  dom_movie=dom_movie, w_q=w_q, w_k=w_k, w_v=w_v, b_q=b_q,
                  b_k=b_k, b_v=b_v, w_o=w_o, b_o=b_o, ln1_g=ln1_g,
                  ln1_b=ln1_b, w1=w1, b1=b1, w2=w2, b2=b2, ln2_g=ln2_g,
                  ln2_b=ln2_b)
    prep, t1, t2 = host_prepare(inputs)
    nc = _get_program(B_CORE, r_blk, t1, t2)

    xc = np.asarray(h_u_cross, np.float32)
    xt = np.asarray(h_u_target, np.float32)
    xtb = np.ascontiguousarray(xt.astype(NPBF16))
    db = np.ascontiguousarray((xc - xt).astype(NPBF16))
    in_maps = []
    for c in range(N_CORES):
        m = dict(prep)
        m["xtb"] = xtb[c * B_CORE:(c + 1) * B_CORE]
        m["db"] = db[c * B_CORE:(c + 1) * B_CORE]
        in_maps.append(m)

    res = run_bass_kernel_spmd(nc, in_maps, core_ids=list(range(N_CORES)),
                               trace=trace, **run_kwargs)
    out = np.concatenate([res.results[c]["out"] for c in range(N_CORES)], axis=0)
    kernel.last_results = res
    return out.astype(np.float32)
